# revision 6
# baseline (speedup 1.0000x reference)
"""Trainium2 Bass kernel for EnergyConstrainedPredictiveCodingModel.

Data-parallel over the batch dim across 8 NeuronCores; weights replicated.

v2 design (from baseline trace analysis: PE 188us busy, DMA ~180us at the
~360GB/s/core ceiling, Scalar 150us, DVE 113us -> everything near-roofline):
  - DMA bytes 64MB -> ~44MB/core: all activations shipped bf16 (packed into
    one [BL,4096] buffer -> 1 input DMA per row-tile), I_t/h/h2 additionally
    shipped host-transposed (fp8/bf16/bf16) killing 16 of 28 PE transposes
    per tile, weights shipped fp8/bf16.
  - PE: fp8e4m3 DoubleRow matmuls (0.5 cyc/row) for the error-tolerant
    matmuls (ith, muq, sq, vip, sst, z->h, z->h2, recon); bf16 (1.0 cyc/row,
    same as f32r) for the l2err-critical sigp/mup and for hh/h2h2 (reuse of
    the bf16 hT/h2T).  Remaining on-chip transposes (sigma_p, theta, z) run
    f32-in with the fp8 cast folded into the PSUM->SBUF evict.
  - Elementwise spread over Scalar (tanh/exp/abs/square/copy: one act-table
    set), DVE, and the otherwise-idle GpSimd; sigmoid via 0.5*tanh(0.5x)+0.5.
  - Outputs assembled in a [128, 6144] staging tile, written with 3 big
    contiguous DMAs per row-tile (z_energy = second DMA of the z column).

Model (per reference):
  B=8192, D=1024, L=512, H=512, REC=256, MAX_NORM=0.5
  out = concat([z, h_new, h2_new, sigma_p, theta, sst_inh, theta_ff,
                z_energy, I_hat, layer_1_error, layer_2_error], -1)
"""

import numpy as np
import ml_dtypes
from contextlib import ExitStack

import concourse.bass as bass
import concourse.mybir as mybir
import concourse.tile as tile
from concourse import bacc
from concourse.bass_utils import run_bass_kernel_spmd
from concourse.masks import make_identity

B, D, L, H, REC = 8192, 1024, 512, 512, 256
MAX_NORM = 0.5
N_CORES = 8
BL = B // N_CORES            # rows per core
P = 128                      # partitions
NT = BL // P                 # row tiles per core
OUT_W = 9 * L + 2 * D        # 6656
PACK_W = D + 6 * L           # 4096 packed bf16 input columns

F32 = mybir.dt.float32
F32R = mybir.dt.float32r
BF16 = mybir.dt.bfloat16
FP8 = mybir.dt.float8e4
AF = mybir.ActivationFunctionType
OP = mybir.AluOpType
DR = mybir.MatmulPerfMode.DoubleRow

NP_BF16 = ml_dtypes.bfloat16
NP_FP8 = ml_dtypes.float8_e4m3

# output column offsets
OFF_Z = 0
OFF_HN = L
OFF_H2N = 2 * L
OFF_SP = 3 * L
OFF_TH = 4 * L
OFF_SST = 5 * L
OFF_TFF = 6 * L
OFF_ZE = 7 * L          # == z, written via a second DMA of the z column
OFF_IH = 8 * L
OFF_L1 = 8 * L + D
OFF_L2 = 8 * L + 2 * D

# staging tile column offsets (no z_energy column; 6144 wide)
ST_W = 7 * L + 2 * D + L     # 6144
ST_IH = 7 * L                # 3584
ST_L1 = 7 * L + D
ST_L2 = 7 * L + 2 * D

# packed input columns
PK_IT = 0
PK_SPP = D
PK_TFFP = D + L
PK_TP = D + 2 * L
PK_SSTP = D + 3 * L
PK_EPSZ = D + 4 * L
PK_EPSZH = D + 5 * L


def _mm16(nc, out_ps, lhsT_sb, rows, w_sb, nk, first=True, last=True):
    """out += lhsT.T @ w over nk 128-chunks; bf16 operands, rows-slice of
    the [P, nk, BL] transposed-activation tile."""
    for c in range(nk):
        nc.tensor.matmul(
            out_ps,
            lhsT_sb[:, c, rows],
            w_sb[:, c, :],
            start=(first and c == 0),
            stop=(last and c == nk - 1),
        )


def _mmdr(nc, out_ps, lhsT_sb, rows, w_sb, npair, first=True, last=True,
          n_slice=None):
    """fp8 DoubleRow: out += lhsT.T @ w over npair k-chunk pairs."""
    for c in range(npair):
        rhs = (w_sb[:, 2 * c:2 * c + 2, :] if n_slice is None
               else w_sb[:, 2 * c:2 * c + 2, n_slice])
        if rows is None:
            lhs = lhsT_sb[:, 2 * c:2 * c + 2, :]
        else:
            lhs = lhsT_sb[:, 2 * c:2 * c + 2, rows]
        nc.tensor.matmul(
            out_ps,
            lhs,
            rhs,
            start=(first and c == 0),
            stop=(last and c == npair - 1),
            perf_mode=DR,
        )


def _build_program(bl=BL):
    nc = bacc.Bacc(trn_type="TRN2", target_bir_lowering=False, debug=False)
    nt = bl // P

    def din(name, shape, dtype):
        return nc.dram_tensor(name, shape, dtype, kind="ExternalInput").ap()

    # per-core activations
    pack_d = din("pack", [bl, PACK_W], BF16)          # it|spp|tffp|tp|sstp|epsz|epszh
    itT_d = din("itT", [D, bl], FP8)                  # I_t shard transposed
    hT_d = din("hT", [H, bl], BF16)
    h2T_d = din("h2T", [H, bl], BF16)
    # weights ([in, out] layout, host-cast)
    wpm_d = din("wpm_t", [D, L], FP8)
    wps_d = din("wps_t", [D, L], FP8)
    wi2t_d = din("wi2t_t", [D, L], FP8)
    wzh_d = din("wzh_t", [L, H], FP8)
    wzh2_d = din("wzh2_t", [L, H], FP8)
    wvip_d = din("wvip_t", [L, L], FP8)
    wt2z_d = din("wt2z_t", [L, L], FP8)
    whh_d = din("whh_t", [H, H], BF16)
    wh2h2_d = din("wh2h2_t", [H, H], BF16)
    wprs_d = din("wprs_t", [H, L], BF16)
    wprm_d = din("wprm_t", [H, L], BF16)
    wrec1_d = din("wrec1", [REC, L], BF16)            # natural [out=REC? no: [REC, L]]
    wrec2_d = din("wrec2_t", [REC, D], BF16)
    bps_d = din("bps", [1, L], F32)

    out_d = nc.dram_tensor("out", [bl, OUT_W], F32, kind="ExternalOutput").ap()

    with tile.TileContext(nc) as tc, ExitStack() as ctx:
        weights = ctx.enter_context(tc.tile_pool(name="weights", bufs=1))
        consts = ctx.enter_context(tc.tile_pool(name="consts", bufs=1))
        psum = ctx.enter_context(tc.tile_pool(name="psum", bufs=8, space="PSUM"))
        pool_in = ctx.enter_context(tc.tile_pool(name="inp", bufs=3))
        pool_tin = ctx.enter_context(tc.tile_pool(name="tin", bufs=1))
        pool_st = ctx.enter_context(tc.tile_pool(name="stage", bufs=2))
        pool_im = ctx.enter_context(tc.tile_pool(name="interm", bufs=2))
        pool_tr = ctx.enter_context(tc.tile_pool(name="trans", bufs=2))

        ident = consts.tile([P, P], F32)
        make_identity(nc, ident)
        ones_row = consts.tile([1, P], F32R)
        onesf = consts.tile([1, P], F32)
        nc.vector.memset(onesf, 1.0)
        nc.scalar.copy(ones_row, onesf)
        ones_L = consts.tile([1, L], F32R)
        onesLf = consts.tile([1, L], F32)
        nc.vector.memset(onesLf, 1.0)
        nc.scalar.copy(ones_L, onesLf)
        ones_col = consts.tile([P, 1], F32)
        nc.vector.memset(ones_col, 1.0)
        neg1_col = consts.tile([P, 1], F32)
        nc.vector.memset(neg1_col, -1.0)
        bps = consts.tile([1, L], F32R)

        # ---- big up-front input DMAs (transposed activations) ----
        itT = pool_tin.tile([P, D // P, bl], FP8, tag="itT")
        nc.sync.dma_start(out=itT, in_=itT_d.rearrange("(c p) n -> p c n", p=P))
        hT = pool_tin.tile([P, H // P, bl], BF16, tag="hT")
        nc.sync.dma_start(out=hT, in_=hT_d.rearrange("(c p) n -> p c n", p=P))
        h2T = pool_tin.tile([P, H // P, bl], BF16, tag="h2T")
        nc.sync.dma_start(out=h2T, in_=h2T_d.rearrange("(c p) n -> p c n", p=P))

        def load_pack(t):
            rows = slice(t * P, (t + 1) * P)
            pk = pool_in.tile([P, PACK_W], BF16, tag="pack", name="pack_sb")
            nc.sync.dma_start(out=pk, in_=pack_d[rows, :])
            return pk

        pk0 = load_pack(0)
        pk1 = load_pack(1)

        def wload(dram_ap, K, N, name, dtype):
            t = weights.tile([P, K // P, N], dtype, tag=name, name=name)
            nc.sync.dma_start(out=t, in_=dram_ap.rearrange("(c p) n -> p c n", p=P))
            return t

        # ---- stage-1 weights, ordered by first use ----
        wprs = wload(wprs_d, H, L, "wprs", BF16)
        bps_st = consts.tile([1, L], F32)
        nc.sync.dma_start(out=bps_st, in_=bps_d)
        nc.scalar.activation(bps, bps_st, AF.Relu)
        wi2t = wload(wi2t_d, D, L, "wi2t", FP8)
        wvip = wload(wvip_d, L, L, "wvip", FP8)
        nc.vector.tensor_scalar_max(
            wvip.rearrange("p c n -> p (c n)"), wvip.rearrange("p c n -> p (c n)"), 0.0
        )
        wprm = wload(wprm_d, H, L, "wprm", BF16)
        wpm = wload(wpm_d, D, L, "wpm", FP8)
        wps = wload(wps_d, D, L, "wps", FP8)
        # ---- tail weights ----
        wt2z = wload(wt2z_d, L, L, "wt2z", FP8)
        nc.vector.tensor_scalar_max(
            wt2z.rearrange("p c n -> p (c n)"), wt2z.rearrange("p c n -> p (c n)"), 0.0
        )
        wzh = wload(wzh_d, L, H, "wzh", FP8)
        wzh2 = wload(wzh2_d, L, H, "wzh2", FP8)
        wh2h2 = wload(wh2h2_d, H, H, "wh2h2", BF16)
        whh = weights.tile([P, H // P, H], BF16, tag="whh")
        wrec = weights.tile([P, L // P, D], FP8, tag="wrec")

        with tc.tile_pool(name="setup", bufs=1) as setup:
            # W_h_to_h spectral clip: W * min(1, MAX_NORM / ||W||_F)
            whh_st = setup.tile([P, H // P, H], BF16, tag="whh_st")
            nc.sync.dma_start(
                out=whh_st, in_=whh_d.rearrange("(c p) n -> p c n", p=P)
            )
            whh_f = whh_st.rearrange("p c n -> p (c n)")
            nchk = (H // P) * H // 512
            acc = setup.tile([P, nchk], F32)
            for j in range(nchk):
                scr = setup.tile([P, 512], F32, tag="scr")
                nc.scalar.activation(
                    scr, whh_f[:, j * 512:(j + 1) * 512], AF.Square,
                    accum_out=acc[:, j:j + 1],
                )
            sq_sum = setup.tile([P, 1], F32)
            nc.vector.tensor_reduce(sq_sum, acc, mybir.AxisListType.X, OP.add)
            nrm2_ps = psum.tile([1, 1], F32, tag="ps", name="nrm2_ps")
            nc.tensor.matmul(nrm2_ps, sq_sum, ones_col, start=True, stop=True)
            nrm = setup.tile([1, 1], F32)
            nc.scalar.activation(nrm, nrm2_ps, AF.Sqrt)
            rn = setup.tile([1, 1], F32)
            nc.vector.reciprocal(rn, nrm)
            scale = setup.tile([1, 1], F32)
            nc.vector.tensor_scalar(scale, rn, MAX_NORM, 1.0, OP.mult, OP.min)
            scale_ps = psum.tile([P, 1], F32, tag="ps", name="scale_ps")
            nc.tensor.matmul(scale_ps, onesf, scale, start=True, stop=True)
            scale_bc = setup.tile([P, 1], F32)
            nc.scalar.copy(scale_bc, scale_ps)
            nc.vector.tensor_scalar(
                whh.rearrange("p c n -> p (c n)"), whh_f, scale_bc, None, OP.mult
            )

            # fuse W_rec = (W_rec2 @ W_rec1).T = W_rec1.T @ W_rec2.T (bf16)
            wrec1 = setup.tile([P, REC // P, L], BF16, tag="wrec1")
            nc.sync.dma_start(
                out=wrec1, in_=wrec1_d.rearrange("(c p) n -> p c n", p=P)
            )
            wrec2 = setup.tile([P, REC // P, D], BF16, tag="wrec2")
            nc.sync.dma_start(
                out=wrec2, in_=wrec2_d.rearrange("(c p) n -> p c n", p=P)
            )
            for m in range(L // P):
                for half in range(2):
                    ps = psum.tile([P, 512], F32, tag="ps")
                    for c in range(REC // P):
                        nc.tensor.matmul(
                            ps,
                            wrec1[:, c, m * P:(m + 1) * P],
                            wrec2[:, c, half * 512:(half + 1) * 512],
                            start=(c == 0),
                            stop=(c == REC // P - 1),
                        )
                    nc.scalar.copy(wrec[:, m, half * 512:(half + 1) * 512], ps)

        # PE transpose src (f32 view) [128, 4*128] -> dst fp8 [128, 4, 128]
        # (the PSUM->SBUF evict casts to fp8)
        def transpose4(nc, dst8, src):
            ps = psum.tile([P, 512], F32, tag="ps")
            for j in range(4):
                nc.tensor.transpose(
                    ps[:, j * P:(j + 1) * P], src[:, j * P:(j + 1) * P], ident
                )
            nc.scalar.copy(dst8.rearrange("p c n -> p (c n)"), ps)

        # ---------------- software-pipelined main loop ----------------
        def stage1(t, pk):
            rows = slice(t * P, (t + 1) * P)
            st = {"pk": pk, "rows": rows}
            stg = pool_st.tile([P, ST_W], F32, tag="stg", name="stg")
            st["stg"] = stg
            it16 = pk[:, PK_IT:PK_IT + D]

            # sigma_p = 0.8*relu(h@Wprs.T + b) + 0.2*spp
            sigp_ps = psum.tile([P, L], F32, tag="ps", name="sigp_ps")
            nc.tensor.matmul(sigp_ps, ones_row, bps, start=True, stop=False)
            _mm16(nc, sigp_ps, hT, rows, wprs, H // P, first=False)
            ith_ps = psum.tile([P, L], F32, tag="ps", name="ith_ps")
            _mmdr(nc, ith_ps, itT, rows, wi2t, D // P // 2)

            sp_st = stg[:, OFF_SP:OFF_SP + L]
            tmp_sp = pool_im.tile([P, L], F32, tag="scr1", name="tmp_sp")
            nc.vector.tensor_scalar(tmp_sp, sigp_ps, 0.0, 0.8, OP.max, OP.mult)
            nc.vector.scalar_tensor_tensor(
                sp_st, pk[:, PK_SPP:PK_SPP + L], 0.2, tmp_sp, OP.mult, OP.add
            )

            # theta_ff = tanh(0.4*tffp + exp(-50|tffp|)*(I@Wi2t.T))^2
            a1 = pool_im.tile([P, L], F32, tag="scr2", name="a1")
            nc.scalar.activation(a1, pk[:, PK_TFFP:PK_TFFP + L], AF.Abs)
            nc.scalar.activation(a1, a1, AF.Exp, scale=-50.0)
            tffm = pool_im.tile([P, L], F32, tag="scr3", name="tffm")
            nc.vector.tensor_mul(tffm, a1, ith_ps)
            nc.vector.scalar_tensor_tensor(
                tffm, pk[:, PK_TFFP:PK_TFFP + L], 0.4, tffm, OP.mult, OP.add
            )
            nc.scalar.activation(tffm, tffm, AF.Tanh)
            nc.scalar.activation(stg[:, OFF_TFF:OFF_TFF + L], tffm, AF.Square)

            # vip chain
            spT = pool_tr.tile([P, L // P, P], FP8, tag="spT", name="spT")
            transpose4(nc, spT, sp_st)
            vip_ps = psum.tile([P, L], F32, tag="ps", name="vip_ps")
            nc.tensor.matmul(vip_ps, ones_row, ones_L, start=True, stop=False)
            _mmdr(nc, vip_ps, spT, None, wvip, L // P // 2, first=False)

            # posterior + prior-mu matmuls
            mup_ps = psum.tile([P, L], F32, tag="ps", name="mup_ps")
            _mm16(nc, mup_ps, h2T, rows, wprm, H // P)
            muq_ps = psum.tile([P, L], F32, tag="ps", name="muq_ps")
            _mmdr(nc, muq_ps, itT, rows, wpm, D // P // 2)
            sq_ps = psum.tile([P, L], F32, tag="ps", name="sq_ps")
            _mmdr(nc, sq_ps, itT, rows, wps, D // P // 2)

            # theta = 0.1*tp + tff * (1/(1 + vip))
            th = pool_im.tile([P, L], F32, tag="scr1", name="th")
            nc.vector.tensor_copy(th, vip_ps)
            nc.vector.reciprocal(th, th)
            nc.vector.scalar_tensor_tensor(
                th, stg[:, OFF_TFF:OFF_TFF + L], 1.0, th, OP.mult, OP.mult
            )
            nc.vector.scalar_tensor_tensor(
                stg[:, OFF_TH:OFF_TH + L], pk[:, PK_TP:PK_TP + L], 0.1, th,
                OP.mult, OP.add,
            )

            # posterior evictions + raw_z (independent of theta/sst)
            mup_sb = pool_im.tile([P, L], F32, tag="mup", name="mup_sb")
            nc.scalar.activation(mup_sb, mup_ps, AF.Relu)
            st["mup"] = mup_sb
            muq_sb = pool_im.tile([P, L], F32, tag="scr2", name="muq_sb")
            nc.scalar.activation(muq_sb, muq_ps, AF.Relu)
            s_sb = pool_im.tile([P, L], F32, tag="scr3", name="s_sb")
            nc.scalar.activation(s_sb, sq_ps, AF.Tanh, scale=0.005)
            rz = pool_im.tile([P, L], F32, tag="rz", name="rz")
            nc.vector.scalar_tensor_tensor(
                rz, s_sb, 0.5, pk[:, PK_EPSZ:PK_EPSZ + L], OP.mult, OP.mult
            )
            nc.vector.tensor_add(rz, rz, muq_sb)
            nc.scalar.activation(rz, rz, AF.Tanh)
            st["rz"] = rz
            return st

        def tail(t, st):
            rows = st["rows"]
            pk, stg = st["pk"], st["stg"]
            it16 = pk[:, PK_IT:PK_IT + D]

            # sst_inh = 0.8*sstp + theta@Wt2z_p.T
            thT = pool_tr.tile([P, L // P, P], FP8, tag="thT", name="thT")
            transpose4(nc, thT, stg[:, OFF_TH:OFF_TH + L])
            sst_ps = psum.tile([P, L], F32, tag="ps", name="sst_ps")
            _mmdr(nc, sst_ps, thT, None, wt2z, L // P // 2)
            sst_st = stg[:, OFF_SST:OFF_SST + L]
            nc.vector.scalar_tensor_tensor(
                sst_st, pk[:, PK_SSTP:PK_SSTP + L], 0.8, sst_ps, OP.mult, OP.add
            )

            # z = relu(raw_z - sst)   (== z_energy)
            za = pool_im.tile([P, L], F32, tag="scr1", name="za")
            nc.vector.tensor_sub(za, st["rz"], sst_st)
            nc.vector.tensor_scalar_max(stg[:, OFF_Z:OFF_Z + L], za, 0.0)
            zT = pool_tr.tile([P, L // P, P], FP8, tag="zT", name="zT")
            transpose4(nc, zT, stg[:, OFF_Z:OFF_Z + L])

            # h_new / h2_new
            hn_ps = psum.tile([P, H], F32, tag="ps", name="hn_ps")
            _mm16(nc, hn_ps, hT, rows, whh, H // P, last=False)
            _mmdr(nc, hn_ps, zT, None, wzh, L // P // 2, first=False)
            nc.scalar.activation(stg[:, OFF_HN:OFF_HN + H], hn_ps, AF.Relu)
            h2n_ps = psum.tile([P, H], F32, tag="ps", name="h2n_ps")
            _mm16(nc, h2n_ps, h2T, rows, wh2h2, H // P, last=False)
            _mmdr(nc, h2n_ps, zT, None, wzh2, L // P // 2, first=False)
            nc.scalar.activation(stg[:, OFF_H2N:OFF_H2N + H], h2n_ps, AF.Relu)

            # I_hat = sigmoid(z @ W_rec.T - 2) = 0.5*tanh(0.25*(..) ... ) trick
            #       = 0.5*tanh(0.5*x - 1) + 0.5 ; layer_1_error on gpsimd
            for half in range(2):
                hsl = slice(half * 512, (half + 1) * 512)
                ih_ps = psum.tile([P, 512], F32, tag="ps", name="ih_ps")
                _mmdr(nc, ih_ps, zT, None, wrec, L // P // 2, n_slice=hsl)
                tsb = pool_im.tile([P, 512], F32, tag="scr2", name="tsb")
                nc.scalar.activation(tsb, ih_ps, AF.Tanh, scale=0.5, bias=neg1_col)
                ih_st = stg[:, ST_IH + half * 512:ST_IH + half * 512 + 512]
                nc.vector.tensor_scalar(ih_st, tsb, 0.5, 0.5, OP.mult, OP.add)
                l1_st = stg[:, ST_L1 + half * 512:ST_L1 + half * 512 + 512]
                l1d = pool_im.tile([P, 512], F32, tag="scr3", name="l1d")
                nc.vector.scalar_tensor_tensor(
                    l1d, ih_st, -1.0, it16[:, hsl], OP.mult, OP.add
                )
                nc.scalar.activation(l1_st, l1d, AF.Square)

            # layer_2_error = (z - mu_p - eps_zhat*sigma_p)^2
            e_sb = pool_im.tile([P, L], F32, tag="scr1", name="e_sb")
            nc.vector.tensor_mul(
                e_sb, pk[:, PK_EPSZH:PK_EPSZH + L], stg[:, OFF_SP:OFF_SP + L]
            )
            d_sb = pool_im.tile([P, L], F32, tag="scr2", name="d_sb")
            nc.vector.tensor_sub(d_sb, stg[:, OFF_Z:OFF_Z + L], st["mup"])
            d2 = pool_im.tile([P, L], F32, tag="scr3", name="d2")
            nc.vector.scalar_tensor_tensor(d2, e_sb, -1.0, d_sb, OP.mult, OP.add)
            nc.scalar.activation(stg[:, ST_L2:ST_L2 + L], d2, AF.Square)

            # output DMAs: [z..tff] | z->z_energy | [ihat, l1, l2]
            nc.sync.dma_start(
                out=out_d[rows, 0:ST_IH], in_=stg[:, 0:ST_IH]
            )
            nc.sync.dma_start(
                out=out_d[rows, OFF_ZE:OFF_ZE + L], in_=stg[:, OFF_Z:OFF_Z + L]
            )
            nc.sync.dma_start(
                out=out_d[rows, OFF_IH:OFF_IH + 2 * D + L],
                in_=stg[:, ST_IH:ST_IH + 2 * D + L],
            )

        states = {}
        for t in range(nt):
            if t == 0:
                pk = pk0
            elif t == 1:
                pk = pk1
            else:
                pk = load_pack(t)
            states[t] = stage1(t, pk)
            if t >= 1:
                tail(t - 1, states.pop(t - 1))
        tail(nt - 1, states.pop(nt - 1))

    nc.compile()
    return nc


_NC_CACHE = []


def _get_program():
    if not _NC_CACHE:
        _NC_CACHE.append(_build_program())
    return _NC_CACHE[0]


def _prep_in_maps(inputs):
    f32 = lambda a: np.asarray(a, dtype=np.float32)
    it = f32(inputs["I_t"]).reshape(N_CORES, BL, D)
    pack = np.concatenate(
        [
            it,
            f32(inputs["sigma_p_prev"]).reshape(N_CORES, BL, L),
            f32(inputs["theta_ff_prev"]).reshape(N_CORES, BL, L),
            f32(inputs["theta_prev"]).reshape(N_CORES, BL, L),
            f32(inputs["sst_inh_prev"]).reshape(N_CORES, BL, L),
            f32(inputs["eps_z"]).reshape(N_CORES, BL, L),
            f32(inputs["eps_zhat"]).reshape(N_CORES, BL, L),
        ],
        axis=2,
    ).astype(NP_BF16)
    itT = np.ascontiguousarray(
        it.transpose(0, 2, 1)).astype(NP_FP8)
    hT = np.ascontiguousarray(
        f32(inputs["h"]).reshape(N_CORES, BL, H).transpose(0, 2, 1)
    ).astype(NP_BF16)
    h2T = np.ascontiguousarray(
        f32(inputs["h2"]).reshape(N_CORES, BL, H).transpose(0, 2, 1)
    ).astype(NP_BF16)

    trc = lambda a, d: np.ascontiguousarray(f32(a).T).astype(d)
    rep = {
        "wpm_t": trc(inputs["W_post_mu"], NP_FP8),
        "wps_t": trc(inputs["W_post_sigma"], NP_FP8),
        "wi2t_t": trc(inputs["W_I_to_theta"], NP_FP8),
        "wzh_t": trc(inputs["W_z_to_h"], NP_FP8),
        "wzh2_t": trc(inputs["W_z_to_h2"], NP_FP8),
        "wvip_t": trc(inputs["W_vip"], NP_FP8),
        "wt2z_t": trc(inputs["W_theta_to_z"], NP_FP8),
        "whh_t": trc(inputs["W_h_to_h"], NP_BF16),
        "wh2h2_t": trc(inputs["W_h2_to_h2"], NP_BF16),
        "wprs_t": trc(inputs["W_prior_sigma"], NP_BF16),
        "wprm_t": trc(inputs["W_prior_mu"], NP_BF16),
        "wrec1": np.ascontiguousarray(f32(inputs["W_rec1"])).astype(NP_BF16),
        "wrec2_t": trc(inputs["W_rec2"], NP_BF16),
        "bps": np.ascontiguousarray(
            f32(inputs["b_prior_sigma"]).reshape(1, L)),
    }
    return [
        {
            "pack": pack[i], "itT": itT[i], "hT": hT[i], "h2T": h2T[i],
            **rep,
        }
        for i in range(N_CORES)
    ]


def run(inputs, trace=False, **kw):
    nc = _get_program()
    in_maps = _prep_in_maps(inputs)
    res = run_bass_kernel_spmd(
        nc, in_maps, core_ids=list(range(N_CORES)), trace=trace, **kw
    )
    out = np.concatenate([res.results[i]["out"] for i in range(N_CORES)], axis=0)
    return out, res


def kernel(**inputs):
    out, _ = run(inputs)
    return out


# revision 8
# speedup vs baseline: 1.1303x; 1.1303x over previous
"""Trainium2 Bass kernel for EnergyConstrainedPredictiveCodingModel.

Data-parallel over the batch dim across 8 NeuronCores; weights replicated.

v3 design (see git history for the baseline):
  - DMA bytes 64MB -> ~44MB/core: activations bf16, split into an S1-pack
    [BL,2048] (spp|tffp|tp|epsz) and a tail-pack [BL,2048] (I_t|sstp|epszh);
    I_t/h/h2 additionally host-transposed (fp8/bf16/bf16); weights fp8/bf16.
  - PE: fp8e4m3 DoubleRow matmuls for ith/muq/sq/vip/sst/zh/zh2/recon; bf16
    for sigp/mup (l2err-critical) and hh/h2h2.  On-chip transposes only for
    sigma_p/theta/z (f32 in, fp8 cast at the PSUM evict).
  - Software pipeline depth 3 (S1(t) ... tail(t-3)) so every engine's
    in-order queue only sees ready instructions; independent matmuls are
    emitted before dependent transposes inside each stage.
  - Elementwise split across ACT (tanh/exp/abs/relu/copy: one table set),
    DVE (stt chains, fast approx reciprocal), Pool/GpSimd (SBUF-only ops).
  - Outputs staged in stgA [128,1536] (sigma_p|theta|theta_ff, from S1) and
    stgB [128,4608] (z|hn|h2n|sst|ihat|l1|l2, from tail); 6 contiguous
    output DMAs per row-tile (z_energy = 2nd DMA of the z column).
"""

import numpy as np
import ml_dtypes
from contextlib import ExitStack

import concourse.bass as bass
import concourse.mybir as mybir
import concourse.tile as tile
from concourse import bacc
from concourse.bass_utils import run_bass_kernel_spmd
from concourse.masks import make_identity

B, D, L, H, REC = 8192, 1024, 512, 512, 256
MAX_NORM = 0.5
N_CORES = 8
BL = B // N_CORES            # rows per core
P = 128                      # partitions
NT = BL // P                 # row tiles per core
OUT_W = 9 * L + 2 * D        # 6656
DEPTH = 3                    # software pipeline depth

F32 = mybir.dt.float32
F32R = mybir.dt.float32r
BF16 = mybir.dt.bfloat16
FP8 = mybir.dt.float8e4
AF = mybir.ActivationFunctionType
OP = mybir.AluOpType
DR = mybir.MatmulPerfMode.DoubleRow

NP_BF16 = ml_dtypes.bfloat16
NP_FP8 = ml_dtypes.float8_e4m3

# output column offsets
OFF_Z = 0
OFF_HN = L
OFF_H2N = 2 * L
OFF_SP = 3 * L
OFF_TH = 4 * L
OFF_SST = 5 * L
OFF_TFF = 6 * L
OFF_ZE = 7 * L
OFF_IH = 8 * L
OFF_L1 = 8 * L + D
OFF_L2 = 8 * L + 2 * D

# stgA columns: [sigma_p | theta | theta_ff]
A_SP, A_TH, A_TFF, A_W = 0, L, 2 * L, 3 * L
# stgB columns: [z | hn | h2n | sst | ihat | l1 | l2]
B_Z, B_HN, B_H2N, B_SST = 0, L, 2 * L, 3 * L
B_IH, B_L1, B_L2 = 4 * L, 4 * L + D, 4 * L + 2 * D
B_W = 4 * L + 2 * D + L      # 4608

# packA columns (S1-only inputs)
PA_SPP, PA_TFFP, PA_TP, PA_EPSZ, PA_W = 0, L, 2 * L, 3 * L, 4 * L
# packB columns (tail-only inputs)
PB_IT, PB_SSTP, PB_EPSZH, PB_W = 0, D, D + L, D + 2 * L


def _mm16(nc, out_ps, lhsT_sb, rows, w_sb, nk, first=True, last=True):
    for c in range(nk):
        nc.tensor.matmul(
            out_ps,
            lhsT_sb[:, c, rows],
            w_sb[:, c, :],
            start=(first and c == 0),
            stop=(last and c == nk - 1),
        )


def _mmdr(nc, out_ps, lhsT_sb, rows, w_sb, npair, first=True, last=True,
          n_slice=None):
    for c in range(npair):
        rhs = (w_sb[:, 2 * c:2 * c + 2, :] if n_slice is None
               else w_sb[:, 2 * c:2 * c + 2, n_slice])
        if rows is None:
            lhs = lhsT_sb[:, 2 * c:2 * c + 2, :]
        else:
            lhs = lhsT_sb[:, 2 * c:2 * c + 2, rows]
        nc.tensor.matmul(
            out_ps, lhs, rhs,
            start=(first and c == 0),
            stop=(last and c == npair - 1),
            perf_mode=DR,
        )


def _build_program(bl=BL):
    nc = bacc.Bacc(trn_type="TRN2", target_bir_lowering=False, debug=False)
    nt = bl // P

    def din(name, shape, dtype):
        return nc.dram_tensor(name, shape, dtype, kind="ExternalInput").ap()

    packa_d = din("packa", [bl, PA_W], BF16)
    packb_d = din("packb", [bl, PB_W], BF16)
    itT_d = din("itT", [D, bl], FP8)
    hT_d = din("hT", [H, bl], BF16)
    h2T_d = din("h2T", [H, bl], BF16)
    wpm_d = din("wpm_t", [D, L], FP8)
    wps_d = din("wps_t", [D, L], FP8)
    wi2t_d = din("wi2t_t", [D, L], FP8)
    wzh_d = din("wzh_t", [L, H], FP8)
    wzh2_d = din("wzh2_t", [L, H], FP8)
    wvip_d = din("wvip_t", [L, L], FP8)
    wt2z_d = din("wt2z_t", [L, L], FP8)
    whh_d = din("whh_t", [H, H], BF16)
    wh2h2_d = din("wh2h2_t", [H, H], BF16)
    wprs_d = din("wprs_t", [H, L], BF16)
    wprm_d = din("wprm_t", [H, L], BF16)
    wrec1_d = din("wrec1", [REC, L], BF16)
    wrec2_d = din("wrec2_t", [REC, D], BF16)
    bps_d = din("bps", [1, L], F32)

    out_d = nc.dram_tensor("out", [bl, OUT_W], F32, kind="ExternalOutput").ap()

    with tile.TileContext(nc) as tc, ExitStack() as ctx:
        weights = ctx.enter_context(tc.tile_pool(name="weights", bufs=1))
        consts = ctx.enter_context(tc.tile_pool(name="consts", bufs=1))
        psum = ctx.enter_context(tc.tile_pool(name="psum", bufs=8, space="PSUM"))
        pool_ina = ctx.enter_context(tc.tile_pool(name="inpa", bufs=3))
        pool_inb = ctx.enter_context(tc.tile_pool(name="inpb", bufs=DEPTH + 1))
        pool_tin = ctx.enter_context(tc.tile_pool(name="tin", bufs=1))
        pool_sa = ctx.enter_context(tc.tile_pool(name="stga", bufs=DEPTH + 1))
        pool_stb = ctx.enter_context(tc.tile_pool(name="stgb", bufs=2))
        pool_s1 = ctx.enter_context(tc.tile_pool(name="s1t", bufs=DEPTH + 1))
        pool_im = ctx.enter_context(tc.tile_pool(name="interm", bufs=2))
        pool_tr = ctx.enter_context(tc.tile_pool(name="trans", bufs=2))

        ident = consts.tile([P, P], F32)
        make_identity(nc, ident)
        ones_row = consts.tile([1, P], F32R)
        onesf = consts.tile([1, P], F32)
        nc.vector.memset(onesf, 1.0)
        nc.scalar.copy(ones_row, onesf)
        ones_col = consts.tile([P, 1], F32)
        nc.vector.memset(ones_col, 1.0)
        neg1_col = consts.tile([P, 1], F32)
        nc.vector.memset(neg1_col, -1.0)
        bps = consts.tile([1, L], F32R)

        # ---- big up-front input DMAs (transposed activations) ----
        itT = pool_tin.tile([P, D // P, bl], FP8, tag="itT")
        nc.sync.dma_start(out=itT, in_=itT_d.rearrange("(c p) n -> p c n", p=P))
        hT = pool_tin.tile([P, H // P, bl], BF16, tag="hT")
        nc.sync.dma_start(out=hT, in_=hT_d.rearrange("(c p) n -> p c n", p=P))
        h2T = pool_tin.tile([P, H // P, bl], BF16, tag="h2T")
        nc.sync.dma_start(out=h2T, in_=h2T_d.rearrange("(c p) n -> p c n", p=P))

        def load_pack(pool, dram, width, t, name):
            rows = slice(t * P, (t + 1) * P)
            pk = pool.tile([P, width], BF16, tag=name, name=name)
            nc.sync.dma_start(out=pk, in_=dram[rows, :])
            return pk

        pka = {t: load_pack(pool_ina, packa_d, PA_W, t, "pka") for t in range(2)}
        pkb = {}

        def wload(dram_ap, K, N, name, dtype):
            t = weights.tile([P, K // P, N], dtype, tag=name, name=name)
            nc.sync.dma_start(out=t, in_=dram_ap.rearrange("(c p) n -> p c n", p=P))
            return t

        wprs = wload(wprs_d, H, L, "wprs", BF16)
        bps_st = consts.tile([1, L], F32)
        nc.sync.dma_start(out=bps_st, in_=bps_d)
        nc.scalar.activation(bps, bps_st, AF.Relu)
        wi2t = wload(wi2t_d, D, L, "wi2t", FP8)
        wvip = wload(wvip_d, L, L, "wvip", FP8)
        nc.vector.tensor_scalar_max(
            wvip.rearrange("p c n -> p (c n)"), wvip.rearrange("p c n -> p (c n)"), 0.0
        )
        wprm = wload(wprm_d, H, L, "wprm", BF16)
        wpm = wload(wpm_d, D, L, "wpm", FP8)
        wps = wload(wps_d, D, L, "wps", FP8)
        wt2z = wload(wt2z_d, L, L, "wt2z", FP8)
        nc.vector.tensor_scalar_max(
            wt2z.rearrange("p c n -> p (c n)"), wt2z.rearrange("p c n -> p (c n)"), 0.0
        )
        wzh = wload(wzh_d, L, H, "wzh", FP8)
        wzh2 = wload(wzh2_d, L, H, "wzh2", FP8)
        wh2h2 = wload(wh2h2_d, H, H, "wh2h2", BF16)
        whh = weights.tile([P, H // P, H], BF16, tag="whh")
        wrec = weights.tile([P, L // P, D], FP8, tag="wrec")

        with tc.tile_pool(name="setup", bufs=1) as setup:
            # W_h_to_h spectral clip: W * min(1, MAX_NORM / ||W||_F)
            whh_st = setup.tile([P, H // P, H], BF16, tag="whh_st")
            nc.sync.dma_start(
                out=whh_st, in_=whh_d.rearrange("(c p) n -> p c n", p=P)
            )
            whh_f = whh_st.rearrange("p c n -> p (c n)")
            nchk = (H // P) * H // 512
            acc = setup.tile([P, nchk], F32)
            for j in range(nchk):
                scr = setup.tile([P, 512], F32, tag="scr")
                nc.scalar.activation(
                    scr, whh_f[:, j * 512:(j + 1) * 512], AF.Square,
                    accum_out=acc[:, j:j + 1],
                )
            sq_sum = setup.tile([P, 1], F32)
            nc.vector.tensor_reduce(sq_sum, acc, mybir.AxisListType.X, OP.add)
            nrm2_ps = psum.tile([1, 1], F32, tag="ps", name="nrm2_ps")
            nc.tensor.matmul(nrm2_ps, sq_sum, ones_col, start=True, stop=True)
            nrm = setup.tile([1, 1], F32)
            nc.scalar.activation(nrm, nrm2_ps, AF.Sqrt)
            rn = setup.tile([1, 1], F32)
            nc.vector.reciprocal(rn, nrm)
            scale = setup.tile([1, 1], F32)
            nc.vector.tensor_scalar(scale, rn, MAX_NORM, 1.0, OP.mult, OP.min)
            scale_ps = psum.tile([P, 1], F32, tag="ps", name="scale_ps")
            nc.tensor.matmul(scale_ps, onesf, scale, start=True, stop=True)
            scale_bc = setup.tile([P, 1], F32)
            nc.scalar.copy(scale_bc, scale_ps)
            nc.vector.tensor_scalar(
                whh.rearrange("p c n -> p (c n)"), whh_f, scale_bc, None, OP.mult
            )

            # fuse W_rec = (W_rec2 @ W_rec1).T = W_rec1.T @ W_rec2.T (bf16)
            wrec1 = setup.tile([P, REC // P, L], BF16, tag="wrec1")
            nc.sync.dma_start(
                out=wrec1, in_=wrec1_d.rearrange("(c p) n -> p c n", p=P)
            )
            wrec2 = setup.tile([P, REC // P, D], BF16, tag="wrec2")
            nc.sync.dma_start(
                out=wrec2, in_=wrec2_d.rearrange("(c p) n -> p c n", p=P)
            )
            for m in range(L // P):
                for half in range(2):
                    ps = psum.tile([P, 512], F32, tag="ps")
                    for c in range(REC // P):
                        nc.tensor.matmul(
                            ps,
                            wrec1[:, c, m * P:(m + 1) * P],
                            wrec2[:, c, half * 512:(half + 1) * 512],
                            start=(c == 0),
                            stop=(c == REC // P - 1),
                        )
                    nc.scalar.copy(wrec[:, m, half * 512:(half + 1) * 512], ps)

        def transpose4(nc, dst8, src):
            ps = psum.tile([P, 512], F32, tag="ps")
            for j in range(4):
                nc.tensor.transpose(
                    ps[:, j * P:(j + 1) * P], src[:, j * P:(j + 1) * P], ident
                )
            nc.scalar.copy(dst8.rearrange("p c n -> p (c n)"), ps)

        # ---------------- software-pipelined main loop ----------------
        def stage1(t, pk):
            rows = slice(t * P, (t + 1) * P)
            st = {"pka": pk, "rows": rows}
            sa = pool_sa.tile([P, A_W], F32, tag="sa", name="sa")
            st["sa"] = sa

            # --- independent matmuls first (PE never head-blocks) ---
            sigp_ps = psum.tile([P, L], F32, tag="ps", name="sigp_ps")
            nc.tensor.matmul(sigp_ps, ones_row, bps, start=True, stop=False)
            _mm16(nc, sigp_ps, hT, rows, wprs, H // P, first=False)
            ith_ps = psum.tile([P, L], F32, tag="ps", name="ith_ps")
            _mmdr(nc, ith_ps, itT, rows, wi2t, D // P // 2)
            mup_ps = psum.tile([P, L], F32, tag="ps", name="mup_ps")
            _mm16(nc, mup_ps, h2T, rows, wprm, H // P)
            muq_ps = psum.tile([P, L], F32, tag="ps", name="muq_ps")
            _mmdr(nc, muq_ps, itT, rows, wpm, D // P // 2)
            sq_ps = psum.tile([P, L], F32, tag="ps", name="sq_ps")
            _mmdr(nc, sq_ps, itT, rows, wps, D // P // 2)

            # sigma_p = 0.8*relu(h@Wprs.T + b) + 0.2*spp
            tmp_sp = pool_im.tile([P, L], F32, tag="scr1", name="tmp_sp")
            nc.scalar.activation(tmp_sp, sigp_ps, AF.Relu, scale=0.8)
            nc.vector.scalar_tensor_tensor(
                sa[:, A_SP:A_SP + L], pk[:, PA_SPP:PA_SPP + L], 0.2, tmp_sp,
                OP.mult, OP.add,
            )

            # theta_ff = tanh(0.4*tffp + exp(-50|tffp|)*(I@Wi2t.T))^2
            a1 = pool_im.tile([P, L], F32, tag="scr2", name="a1")
            nc.scalar.activation(a1, pk[:, PA_TFFP:PA_TFFP + L], AF.Abs)
            nc.scalar.activation(a1, a1, AF.Exp, scale=-50.0)
            tffm = pool_im.tile([P, L], F32, tag="scr3", name="tffm")
            nc.vector.tensor_mul(tffm, a1, ith_ps)
            nc.vector.scalar_tensor_tensor(
                tffm, pk[:, PA_TFFP:PA_TFFP + L], 0.4, tffm, OP.mult, OP.add
            )
            nc.scalar.activation(tffm, tffm, AF.Tanh)
            nc.gpsimd.tensor_mul(sa[:, A_TFF:A_TFF + L], tffm, tffm)

            # mu_p (held in SBUF for the tail's l2err)
            mup_sb = pool_s1.tile([P, L], F32, tag="mup", name="mup_sb")
            nc.scalar.activation(mup_sb, mup_ps, AF.Relu)
            st["mup"] = mup_sb

            # raw_z = tanh(relu(muq) + eps_z*0.5*tanh(0.005*sq))
            s_sb = pool_im.tile([P, L], F32, tag="scr1", name="s_sb")
            nc.scalar.activation(s_sb, sq_ps, AF.Tanh, scale=0.005)
            rz = pool_s1.tile([P, L], F32, tag="rz", name="rz")
            nc.vector.scalar_tensor_tensor(
                rz, s_sb, 0.5, pk[:, PA_EPSZ:PA_EPSZ + L], OP.mult, OP.mult
            )
            nc.vector.scalar_tensor_tensor(
                rz, muq_ps, 0.0, rz, OP.max, OP.add
            )
            nc.scalar.activation(rz, rz, AF.Tanh)
            st["rz"] = rz

            # --- dependent: sigma_p transpose + vip matmul + theta ---
            spT = pool_tr.tile([P, L // P, P], FP8, tag="spT", name="spT")
            transpose4(nc, spT, sa[:, A_SP:A_SP + L])
            vip_ps = psum.tile([P, L], F32, tag="ps", name="vip_ps")
            _mmdr(nc, vip_ps, spT, None, wvip, L // P // 2)

            # theta = 0.1*tp + tff/(1 + vip)
            th = pool_im.tile([P, L], F32, tag="scr2", name="th")
            nc.vector.tensor_scalar_add(th, vip_ps, 1.0)
            nc.vector.reciprocal_approx_fast(out=th, in_=th)
            nc.vector.scalar_tensor_tensor(
                th, sa[:, A_TFF:A_TFF + L], 1.0, th, OP.mult, OP.mult
            )
            nc.vector.scalar_tensor_tensor(
                sa[:, A_TH:A_TH + L], pk[:, PA_TP:PA_TP + L], 0.1, th,
                OP.mult, OP.add,
            )
            return st

        def tail(t, st, pk):
            rows = st["rows"]
            sa = st["sa"]
            sb = pool_stb.tile([P, B_W], F32, tag="sb", name="sb")
            it16 = pk[:, PB_IT:PB_IT + D]

            # theta transpose (ready), then hh/h2h2 matmuls cover the evict
            thT = pool_tr.tile([P, L // P, P], FP8, tag="thT", name="thT")
            transpose4(nc, thT, sa[:, A_TH:A_TH + L])
            hn_ps = psum.tile([P, H], F32, tag="ps", name="hn_ps")
            _mm16(nc, hn_ps, hT, rows, whh, H // P, last=False)
            h2n_ps = psum.tile([P, H], F32, tag="ps", name="h2n_ps")
            _mm16(nc, h2n_ps, h2T, rows, wh2h2, H // P, last=False)

            # sst_inh = 0.8*sstp + theta@Wt2z_p.T
            sst_ps = psum.tile([P, L], F32, tag="ps", name="sst_ps")
            _mmdr(nc, sst_ps, thT, None, wt2z, L // P // 2)
            sst_st = sb[:, B_SST:B_SST + L]
            nc.vector.scalar_tensor_tensor(
                sst_st, pk[:, PB_SSTP:PB_SSTP + L], 0.8, sst_ps, OP.mult, OP.add
            )

            # z = relu(raw_z - sst)
            zsub = pool_im.tile([P, L], F32, tag="scr1", name="zsub")
            nc.vector.scalar_tensor_tensor(
                zsub, sst_st, -1.0, st["rz"], OP.mult, OP.add
            )
            nc.gpsimd.tensor_relu(sb[:, B_Z:B_Z + L], zsub)
            zT = pool_tr.tile([P, L // P, P], FP8, tag="zT", name="zT")
            transpose4(nc, zT, sb[:, B_Z:B_Z + L])

            # h_new / h2_new (finish the open accumulations)
            _mmdr(nc, hn_ps, zT, None, wzh, L // P // 2, first=False)
            nc.scalar.activation(sb[:, B_HN:B_HN + H], hn_ps, AF.Relu)
            _mmdr(nc, h2n_ps, zT, None, wzh2, L // P // 2, first=False)
            nc.scalar.activation(sb[:, B_H2N:B_H2N + H], h2n_ps, AF.Relu)

            # I_hat = sigmoid(z@W_rec.T - 2) = 0.5*tanh(0.5*x - 1) + 0.5
            for half in range(2):
                hsl = slice(half * 512, (half + 1) * 512)
                ih_ps = psum.tile([P, 512], F32, tag="ps", name="ih_ps")
                _mmdr(nc, ih_ps, zT, None, wrec, L // P // 2, n_slice=hsl)
                tsb = pool_im.tile([P, 512], F32, tag="scr2", name="tsb")
                nc.scalar.activation(tsb, ih_ps, AF.Tanh, scale=0.5, bias=neg1_col)
                ih_st = sb[:, B_IH + half * 512:B_IH + half * 512 + 512]
                nc.gpsimd.tensor_scalar(ih_st, tsb, 0.5, 0.5, OP.mult, OP.add)
                l1d = pool_im.tile([P, 512], F32, tag="scr3", name="l1d")
                nc.vector.scalar_tensor_tensor(
                    l1d, ih_st, -1.0, it16[:, hsl], OP.mult, OP.add
                )
                nc.gpsimd.tensor_mul(
                    sb[:, B_L1 + half * 512:B_L1 + half * 512 + 512], l1d, l1d
                )

            # layer_2_error = (z - mu_p - eps_zhat*sigma_p)^2
            e_sb = pool_im.tile([P, L], F32, tag="scr1", name="e_sb")
            nc.vector.tensor_mul(
                e_sb, pk[:, PB_EPSZH:PB_EPSZH + L], sa[:, A_SP:A_SP + L]
            )
            d_sb = pool_im.tile([P, L], F32, tag="scr2", name="d_sb")
            nc.vector.tensor_sub(d_sb, sb[:, B_Z:B_Z + L], st["mup"])
            d2 = pool_im.tile([P, L], F32, tag="scr3", name="d2")
            nc.vector.scalar_tensor_tensor(d2, e_sb, -1.0, d_sb, OP.mult, OP.add)
            nc.gpsimd.tensor_mul(sb[:, B_L2:B_L2 + L], d2, d2)

            # output DMAs (contiguous blocks)
            nc.sync.dma_start(out=out_d[rows, OFF_SP:OFF_SP + 2 * L],
                              in_=sa[:, A_SP:A_SP + 2 * L])
            nc.sync.dma_start(out=out_d[rows, OFF_TFF:OFF_TFF + L],
                              in_=sa[:, A_TFF:A_TFF + L])
            nc.sync.dma_start(out=out_d[rows, OFF_Z:OFF_Z + 3 * L],
                              in_=sb[:, B_Z:B_Z + 3 * L])
            nc.sync.dma_start(out=out_d[rows, OFF_SST:OFF_SST + L],
                              in_=sb[:, B_SST:B_SST + L])
            nc.sync.dma_start(out=out_d[rows, OFF_ZE:OFF_ZE + L],
                              in_=sb[:, B_Z:B_Z + L])
            nc.sync.dma_start(out=out_d[rows, OFF_IH:OFF_IH + 2 * D + L],
                              in_=sb[:, B_IH:B_IH + 2 * D + L])

        states = {}
        for t in range(nt):
            pkb[t] = load_pack(pool_inb, packb_d, PB_W, t, "pkb")
            states[t] = stage1(t, pka.pop(t))
            if t >= DEPTH:
                tail(t - DEPTH, states.pop(t - DEPTH), pkb.pop(t - DEPTH))
            if t + 2 < nt and (t + 2) not in pka:
                pka[t + 2] = load_pack(pool_ina, packa_d, PA_W, t + 2, "pka")
        for t in range(nt - DEPTH, nt):
            tail(t, states.pop(t), pkb.pop(t))

    nc.compile()
    return nc


_NC_CACHE = []


def _get_program():
    if not _NC_CACHE:
        _NC_CACHE.append(_build_program())
    return _NC_CACHE[0]


def _prep_in_maps(inputs):
    f32 = lambda a: np.asarray(a, dtype=np.float32)
    it = f32(inputs["I_t"]).reshape(N_CORES, BL, D)
    packa = np.concatenate(
        [
            f32(inputs["sigma_p_prev"]).reshape(N_CORES, BL, L),
            f32(inputs["theta_ff_prev"]).reshape(N_CORES, BL, L),
            f32(inputs["theta_prev"]).reshape(N_CORES, BL, L),
            f32(inputs["eps_z"]).reshape(N_CORES, BL, L),
        ],
        axis=2,
    ).astype(NP_BF16)
    packb = np.concatenate(
        [
            it,
            f32(inputs["sst_inh_prev"]).reshape(N_CORES, BL, L),
            f32(inputs["eps_zhat"]).reshape(N_CORES, BL, L),
        ],
        axis=2,
    ).astype(NP_BF16)
    itT = np.ascontiguousarray(it.transpose(0, 2, 1)).astype(NP_FP8)
    hT = np.ascontiguousarray(
        f32(inputs["h"]).reshape(N_CORES, BL, H).transpose(0, 2, 1)
    ).astype(NP_BF16)
    h2T = np.ascontiguousarray(
        f32(inputs["h2"]).reshape(N_CORES, BL, H).transpose(0, 2, 1)
    ).astype(NP_BF16)

    trc = lambda a, d: np.ascontiguousarray(f32(a).T).astype(d)
    rep = {
        "wpm_t": trc(inputs["W_post_mu"], NP_FP8),
        "wps_t": trc(inputs["W_post_sigma"], NP_FP8),
        "wi2t_t": trc(inputs["W_I_to_theta"], NP_FP8),
        "wzh_t": trc(inputs["W_z_to_h"], NP_FP8),
        "wzh2_t": trc(inputs["W_z_to_h2"], NP_FP8),
        "wvip_t": trc(inputs["W_vip"], NP_FP8),
        "wt2z_t": trc(inputs["W_theta_to_z"], NP_FP8),
        "whh_t": trc(inputs["W_h_to_h"], NP_BF16),
        "wh2h2_t": trc(inputs["W_h2_to_h2"], NP_BF16),
        "wprs_t": trc(inputs["W_prior_sigma"], NP_BF16),
        "wprm_t": trc(inputs["W_prior_mu"], NP_BF16),
        "wrec1": np.ascontiguousarray(f32(inputs["W_rec1"])).astype(NP_BF16),
        "wrec2_t": trc(inputs["W_rec2"], NP_BF16),
        "bps": np.ascontiguousarray(f32(inputs["b_prior_sigma"]).reshape(1, L)),
    }
    return [
        {"packa": packa[i], "packb": packb[i], "itT": itT[i], "hT": hT[i],
         "h2T": h2T[i], **rep}
        for i in range(N_CORES)
    ]


def run(inputs, trace=False, **kw):
    nc = _get_program()
    in_maps = _prep_in_maps(inputs)
    res = run_bass_kernel_spmd(
        nc, in_maps, core_ids=list(range(N_CORES)), trace=trace, **kw
    )
    out = np.concatenate([res.results[i]["out"] for i in range(N_CORES)], axis=0)
    return out, res


def kernel(**inputs):
    out, _ = run(inputs)
    return out


# revision 11
# speedup vs baseline: 1.2277x; 1.0862x over previous
"""Trainium2 Bass kernel for EnergyConstrainedPredictiveCodingModel.

Data-parallel over the batch dim across 8 NeuronCores; weights replicated.

v3 design (see git history for the baseline):
  - DMA bytes 64MB -> ~44MB/core: activations bf16, split into an S1-pack
    [BL,2048] (spp|tffp|tp|epsz) and a tail-pack [BL,2048] (I_t|sstp|epszh);
    I_t/h/h2 additionally host-transposed (fp8/bf16/bf16); weights fp8/bf16.
  - PE: fp8e4m3 DoubleRow matmuls for ith/muq/sq/vip/sst/zh/zh2/recon; bf16
    for sigp/mup (l2err-critical) and hh/h2h2.  On-chip transposes only for
    sigma_p/theta/z (f32 in, fp8 cast at the PSUM evict).
  - Software pipeline depth 3 (S1(t) ... tail(t-3)) so every engine's
    in-order queue only sees ready instructions; independent matmuls are
    emitted before dependent transposes inside each stage.
  - Elementwise split across ACT (tanh/exp/abs/relu/copy: one table set),
    DVE (stt chains, fast approx reciprocal), Pool/GpSimd (SBUF-only ops).
  - Outputs staged in stgA [128,1536] (sigma_p|theta|theta_ff, from S1) and
    stgB [128,4608] (z|hn|h2n|sst|ihat|l1|l2, from tail); 6 contiguous
    output DMAs per row-tile (z_energy = 2nd DMA of the z column).
"""

import numpy as np
import ml_dtypes
from contextlib import ExitStack

import concourse.bass as bass
import concourse.mybir as mybir
import concourse.tile as tile
from concourse import bacc
from concourse.bass_utils import run_bass_kernel_spmd
from concourse.masks import make_identity

B, D, L, H, REC = 8192, 1024, 512, 512, 256
MAX_NORM = 0.5
N_CORES = 8
BL = B // N_CORES            # rows per core
P = 128                      # partitions
NT = BL // P                 # row tiles per core
OUT_W = 9 * L + 2 * D        # 6656
DEPTH = 3                    # software pipeline depth

F32 = mybir.dt.float32
F32R = mybir.dt.float32r
BF16 = mybir.dt.bfloat16
FP8 = mybir.dt.float8e4
AF = mybir.ActivationFunctionType
OP = mybir.AluOpType
DR = mybir.MatmulPerfMode.DoubleRow

NP_BF16 = ml_dtypes.bfloat16
NP_FP8 = ml_dtypes.float8_e4m3

# output column offsets
OFF_Z = 0
OFF_HN = L
OFF_H2N = 2 * L
OFF_SP = 3 * L
OFF_TH = 4 * L
OFF_SST = 5 * L
OFF_TFF = 6 * L
OFF_ZE = 7 * L
OFF_IH = 8 * L
OFF_L1 = 8 * L + D
OFF_L2 = 8 * L + 2 * D

# stgA columns: [sigma_p | theta | theta_ff]
A_SP, A_TH, A_TFF, A_W = 0, L, 2 * L, 3 * L
# stgB columns: [z | hn | h2n | sst | ihat | l1 | l2]
B_Z, B_HN, B_H2N, B_SST = 0, L, 2 * L, 3 * L
B_IH, B_L1, B_L2 = 4 * L, 4 * L + D, 4 * L + 2 * D
B_W = 4 * L + 2 * D + L      # 4608

# packA columns (S1-only inputs)
PA_SPP, PA_TFFP, PA_TP, PA_EPSZ, PA_W = 0, L, 2 * L, 3 * L, 4 * L
# packB columns (tail-only inputs)
PB_IT, PB_SSTP, PB_EPSZH, PB_W = 0, D, D + L, D + 2 * L


def _mm16(nc, out_ps, lhsT_sb, rows, w_sb, nk, first=True, last=True):
    for c in range(nk):
        nc.tensor.matmul(
            out_ps,
            lhsT_sb[:, c, rows],
            w_sb[:, c, :],
            start=(first and c == 0),
            stop=(last and c == nk - 1),
        )


def _mmdr(nc, out_ps, lhsT_sb, rows, w_sb, npair, first=True, last=True,
          n_slice=None):
    for c in range(npair):
        rhs = (w_sb[:, 2 * c:2 * c + 2, :] if n_slice is None
               else w_sb[:, 2 * c:2 * c + 2, n_slice])
        if rows is None:
            lhs = lhsT_sb[:, 2 * c:2 * c + 2, :]
        else:
            lhs = lhsT_sb[:, 2 * c:2 * c + 2, rows]
        nc.tensor.matmul(
            out_ps, lhs, rhs,
            start=(first and c == 0),
            stop=(last and c == npair - 1),
            perf_mode=DR,
        )


def _build_program(bl=BL):
    nc = bacc.Bacc(trn_type="TRN2", target_bir_lowering=False, debug=False)
    nt = bl // P

    def din(name, shape, dtype):
        return nc.dram_tensor(name, shape, dtype, kind="ExternalInput").ap()

    packa_d = din("packa", [bl, PA_W], BF16)
    packb_d = din("packb", [bl, PB_W], BF16)
    itT_d = din("itT", [D, bl], FP8)
    hT_d = din("hT", [H, bl], BF16)
    h2T_d = din("h2T", [H, bl], BF16)
    wpm_d = din("wpm_t", [D, L], FP8)
    wps_d = din("wps_t", [D, L], FP8)
    wi2t_d = din("wi2t_t", [D, L], FP8)
    wzh_d = din("wzh_t", [L, H], FP8)
    wzh2_d = din("wzh2_t", [L, H], FP8)
    wvip_d = din("wvip_t", [L, L], FP8)
    wt2z_d = din("wt2z_t", [L, L], FP8)
    whh_d = din("whh_t", [H, H], BF16)
    wh2h2_d = din("wh2h2_t", [H, H], BF16)
    wprs_d = din("wprs_t", [H, L], BF16)
    wprm_d = din("wprm_t", [H, L], BF16)
    wrec1_d = din("wrec1", [REC, L], BF16)
    wrec2_d = din("wrec2_t", [REC, D], BF16)
    bps_d = din("bps", [1, L], F32)

    out_d = nc.dram_tensor("out", [bl, OUT_W], F32, kind="ExternalOutput").ap()

    with tile.TileContext(nc) as tc, ExitStack() as ctx:
        weights = ctx.enter_context(tc.tile_pool(name="weights", bufs=1))
        consts = ctx.enter_context(tc.tile_pool(name="consts", bufs=1))
        psum = ctx.enter_context(tc.tile_pool(name="psum", bufs=8, space="PSUM"))
        pool_ina = ctx.enter_context(tc.tile_pool(name="inpa", bufs=3))
        pool_tin = ctx.enter_context(tc.tile_pool(name="tin", bufs=1))

        ident = consts.tile([P, P], F32)
        make_identity(nc, ident)
        ones_row = consts.tile([1, P], F32R)
        onesf = consts.tile([1, P], F32)
        nc.vector.memset(onesf, 1.0)
        nc.scalar.copy(ones_row, onesf)
        ones_col = consts.tile([P, 1], F32)
        nc.vector.memset(ones_col, 1.0)
        neg1_col = consts.tile([P, 1], F32)
        nc.vector.memset(neg1_col, -1.0)
        bps = consts.tile([1, L], F32R)

        # ---- big up-front input DMAs (transposed activations) ----
        itT = pool_tin.tile([P, D // P, bl], FP8, tag="itT")
        nc.sync.dma_start(out=itT, in_=itT_d.rearrange("(c p) n -> p c n", p=P))
        hT = pool_tin.tile([P, H // P, bl], BF16, tag="hT")
        nc.sync.dma_start(out=hT, in_=hT_d.rearrange("(c p) n -> p c n", p=P))
        h2T = pool_tin.tile([P, H // P, bl], BF16, tag="h2T")
        nc.sync.dma_start(out=h2T, in_=h2T_d.rearrange("(c p) n -> p c n", p=P))

        def load_pack(pool, dram, width, t, name):
            rows = slice(t * P, (t + 1) * P)
            pk = pool.tile([P, width], BF16, tag=name, name=name)
            nc.sync.dma_start(out=pk, in_=dram[rows, :])
            return pk

        pka = {t: load_pack(pool_ina, packa_d, PA_W, t, "pka") for t in range(2)}
        pkb = {}

        def wload(dram_ap, K, N, name, dtype):
            t = weights.tile([P, K // P, N], dtype, tag=name, name=name)
            nc.sync.dma_start(out=t, in_=dram_ap.rearrange("(c p) n -> p c n", p=P))
            return t

        wprs = wload(wprs_d, H, L, "wprs", BF16)
        bps_st = consts.tile([1, L], F32)
        nc.sync.dma_start(out=bps_st, in_=bps_d)
        nc.scalar.activation(bps, bps_st, AF.Relu)
        wi2t = wload(wi2t_d, D, L, "wi2t", FP8)
        wvip = wload(wvip_d, L, L, "wvip", FP8)
        nc.vector.tensor_scalar_max(
            wvip.rearrange("p c n -> p (c n)"), wvip.rearrange("p c n -> p (c n)"), 0.0
        )
        wprm = wload(wprm_d, H, L, "wprm", BF16)
        wpm = wload(wpm_d, D, L, "wpm", FP8)
        wps = wload(wps_d, D, L, "wps", FP8)
        wt2z = wload(wt2z_d, L, L, "wt2z", FP8)
        nc.vector.tensor_scalar_max(
            wt2z.rearrange("p c n -> p (c n)"), wt2z.rearrange("p c n -> p (c n)"), 0.0
        )
        wzh = wload(wzh_d, L, H, "wzh", FP8)
        wzh2 = wload(wzh2_d, L, H, "wzh2", FP8)
        wh2h2 = wload(wh2h2_d, H, H, "wh2h2", BF16)
        whh = weights.tile([P, H // P, H], BF16, tag="whh")
        wrec = weights.tile([P, L // P, D], FP8, tag="wrec")

        with tc.tile_pool(name="setup", bufs=1) as setup:
            # W_h_to_h spectral clip: W * min(1, MAX_NORM / ||W||_F)
            # (loaded straight into the final tile; scaled in place)
            nc.sync.dma_start(
                out=whh, in_=whh_d.rearrange("(c p) n -> p c n", p=P)
            )
            whh_f = whh.rearrange("p c n -> p (c n)")
            nchk = (H // P) * H // 512
            acc = setup.tile([P, nchk], F32)
            for j in range(nchk):
                scr = setup.tile([P, 512], F32, tag="scr")
                nc.scalar.activation(
                    scr, whh_f[:, j * 512:(j + 1) * 512], AF.Square,
                    accum_out=acc[:, j:j + 1],
                )
            sq_sum = setup.tile([P, 1], F32)
            nc.vector.tensor_reduce(sq_sum, acc, mybir.AxisListType.X, OP.add)
            nrm2_ps = psum.tile([1, 1], F32, tag="s1ps", bufs=5, name="nrm2_ps")
            nc.tensor.matmul(nrm2_ps, sq_sum, ones_col, start=True, stop=True)
            nrm = setup.tile([1, 1], F32)
            nc.scalar.activation(nrm, nrm2_ps, AF.Sqrt)
            rn = setup.tile([1, 1], F32)
            nc.vector.reciprocal(rn, nrm)
            scale = setup.tile([1, 1], F32)
            nc.vector.tensor_scalar(scale, rn, MAX_NORM, 1.0, OP.mult, OP.min)
            scale_ps = psum.tile([P, 1], F32, tag="s1ps", bufs=5, name="scale_ps")
            nc.tensor.matmul(scale_ps, onesf, scale, start=True, stop=True)
            scale_bc = setup.tile([P, 1], F32)
            nc.scalar.copy(scale_bc, scale_ps)
            nc.vector.tensor_scalar(whh_f, whh_f, scale_bc, None, OP.mult)

            # fuse W_rec = (W_rec2 @ W_rec1).T = W_rec1.T @ W_rec2.T (bf16)
            wrec1 = setup.tile([P, REC // P, L], BF16, tag="wrec1")
            nc.sync.dma_start(
                out=wrec1, in_=wrec1_d.rearrange("(c p) n -> p c n", p=P)
            )
            wrec2 = setup.tile([P, REC // P, D], BF16, tag="wrec2")
            nc.sync.dma_start(
                out=wrec2, in_=wrec2_d.rearrange("(c p) n -> p c n", p=P)
            )
            for m in range(L // P):
                for half in range(2):
                    ps = psum.tile([P, 512], F32, tag="tlps", bufs=3)
                    for c in range(REC // P):
                        nc.tensor.matmul(
                            ps,
                            wrec1[:, c, m * P:(m + 1) * P],
                            wrec2[:, c, half * 512:(half + 1) * 512],
                            start=(c == 0),
                            stop=(c == REC // P - 1),
                        )
                    nc.scalar.copy(wrec[:, m, half * 512:(half + 1) * 512], ps)

        pool_inb = ctx.enter_context(tc.tile_pool(name="inpb", bufs=DEPTH + 1))
        pool_sa = ctx.enter_context(tc.tile_pool(name="stga", bufs=DEPTH + 1))
        pool_stb = ctx.enter_context(tc.tile_pool(name="stgb", bufs=2))
        pool_s1 = ctx.enter_context(tc.tile_pool(name="s1t", bufs=DEPTH + 1))
        pool_im = ctx.enter_context(tc.tile_pool(name="interm", bufs=2))
        pool_tr = ctx.enter_context(tc.tile_pool(name="trans", bufs=2))

        def transpose4(nc, dst8, src, ps):
            for j in range(4):
                nc.tensor.transpose(
                    ps[:, j * P:(j + 1) * P], src[:, j * P:(j + 1) * P], ident
                )
            nc.scalar.copy(dst8.rearrange("p c n -> p (c n)"), ps)

        # ---------------- software-pipelined main loop ----------------
        def stage1(t, pk):
            rows = slice(t * P, (t + 1) * P)
            st = {"pka": pk, "rows": rows}
            sa = pool_sa.tile([P, A_W], F32, tag="sa", name="sa")
            st["sa"] = sa

            # --- independent matmuls first (PE never head-blocks) ---
            # PSUM: 5 banks; sigp's bank is reused by vip, ith's by the
            # sigma_p transpose.
            sigp_ps = psum.tile([P, L], F32, tag="s1ps", bufs=5, name="sigp_ps")
            nc.tensor.matmul(sigp_ps, ones_row, bps, start=True, stop=False)
            _mm16(nc, sigp_ps, hT, rows, wprs, H // P, first=False)
            ith_ps = psum.tile([P, L], F32, tag="s1ps", bufs=5, name="ith_ps")
            _mmdr(nc, ith_ps, itT, rows, wi2t, D // P // 2)
            mup_ps = psum.tile([P, L], F32, tag="s1ps", bufs=5, name="mup_ps")
            _mm16(nc, mup_ps, h2T, rows, wprm, H // P)
            muq_ps = psum.tile([P, L], F32, tag="s1ps", bufs=5, name="muq_ps")
            _mmdr(nc, muq_ps, itT, rows, wpm, D // P // 2)
            sq_ps = psum.tile([P, L], F32, tag="s1ps", bufs=5, name="sq_ps")
            _mmdr(nc, sq_ps, itT, rows, wps, D // P // 2)

            # sigma_p = 0.8*relu(h@Wprs.T + b) + 0.2*spp
            tmp_sp = pool_im.tile([P, L], F32, tag="scr1", bufs=4, name="tmp_sp")
            nc.scalar.activation(tmp_sp, sigp_ps, AF.Relu, scale=0.8)
            nc.vector.scalar_tensor_tensor(
                sa[:, A_SP:A_SP + L], pk[:, PA_SPP:PA_SPP + L], 0.2, tmp_sp,
                OP.mult, OP.add,
            )

            # theta_ff = tanh(0.4*tffp + exp(-50|tffp|)*(I@Wi2t.T))^2
            a1 = pool_im.tile([P, L], F32, tag="scr2", bufs=5, name="a1")
            nc.scalar.activation(a1, pk[:, PA_TFFP:PA_TFFP + L], AF.Abs)
            nc.scalar.activation(a1, a1, AF.Exp, scale=-50.0)
            tffm = pool_im.tile([P, L], F32, tag="scr3", bufs=4, name="tffm")
            nc.vector.tensor_mul(tffm, a1, ith_ps)
            nc.vector.scalar_tensor_tensor(
                tffm, pk[:, PA_TFFP:PA_TFFP + L], 0.4, tffm, OP.mult, OP.add
            )
            nc.scalar.activation(tffm, tffm, AF.Tanh)
            nc.scalar.activation(sa[:, A_TFF:A_TFF + L], tffm, AF.Square)

            # mu_p (held in SBUF for the tail's l2err)
            mup_sb = pool_s1.tile([P, L], F32, tag="mup", name="mup_sb")
            nc.scalar.activation(mup_sb, mup_ps, AF.Relu)
            st["mup"] = mup_sb

            # raw_z = tanh(relu(muq) + eps_z*0.5*tanh(0.005*sq))
            s_sb = pool_im.tile([P, L], F32, tag="scr1", bufs=4, name="s_sb")
            nc.scalar.activation(s_sb, sq_ps, AF.Tanh, scale=0.005)
            rz = pool_s1.tile([P, L], F32, tag="rz", name="rz")
            nc.vector.scalar_tensor_tensor(
                rz, s_sb, 0.5, pk[:, PA_EPSZ:PA_EPSZ + L], OP.mult, OP.mult
            )
            nc.vector.scalar_tensor_tensor(
                rz, muq_ps, 0.0, rz, OP.max, OP.add
            )
            nc.scalar.activation(rz, rz, AF.Tanh)
            st["rz"] = rz

            # --- dependent: sigma_p transpose + vip matmul + theta ---
            # (transpose reuses ith's bank, vip reuses sigp's bank)
            spT = pool_tr.tile([P, L // P, P], FP8, tag="spT", name="spT")
            transpose4(nc, spT, sa[:, A_SP:A_SP + L], ith_ps)
            _mmdr(nc, sigp_ps, spT, None, wvip, L // P // 2)

            # theta = 0.1*tp + tff/(1 + vip)
            th = pool_im.tile([P, L], F32, tag="scr2", bufs=5, name="th")
            nc.vector.tensor_scalar_add(th, sigp_ps, 1.0)
            nc.vector.reciprocal_approx_fast(out=th, in_=th)
            nc.vector.scalar_tensor_tensor(
                th, sa[:, A_TFF:A_TFF + L], 1.0, th, OP.mult, OP.mult
            )
            nc.vector.scalar_tensor_tensor(
                sa[:, A_TH:A_TH + L], pk[:, PA_TP:PA_TP + L], 0.1, th,
                OP.mult, OP.add,
            )
            return st

        def tail(t, st, pk):
            rows = st["rows"]
            sa = st["sa"]
            sb = pool_stb.tile([P, B_W], F32, tag="sb", name="sb")
            it16 = pk[:, PB_IT:PB_IT + D]

            # PSUM: 3 banks; bank A: thT -> sst -> zT; bank B: hn -> ih0;
            # bank C: h2n -> ih1.
            psA = psum.tile([P, L], F32, tag="tlps", bufs=3, name="psA")
            thT = pool_tr.tile([P, L // P, P], FP8, tag="thT", name="thT")
            transpose4(nc, thT, sa[:, A_TH:A_TH + L], psA)
            hn_ps = psum.tile([P, H], F32, tag="tlps", bufs=3, name="hn_ps")
            _mm16(nc, hn_ps, hT, rows, whh, H // P, last=False)
            h2n_ps = psum.tile([P, H], F32, tag="tlps", bufs=3, name="h2n_ps")
            _mm16(nc, h2n_ps, h2T, rows, wh2h2, H // P, last=False)

            # sst_inh = 0.8*sstp + theta@Wt2z_p.T   (bank A again)
            _mmdr(nc, psA, thT, None, wt2z, L // P // 2)
            sst_st = sb[:, B_SST:B_SST + L]
            nc.vector.scalar_tensor_tensor(
                sst_st, pk[:, PB_SSTP:PB_SSTP + L], 0.8, psA, OP.mult, OP.add
            )

            # z = relu(raw_z - sst)
            zsub = pool_im.tile([P, L], F32, tag="scr1", bufs=4, name="zsub")
            nc.vector.scalar_tensor_tensor(
                zsub, sst_st, -1.0, st["rz"], OP.mult, OP.add
            )
            nc.gpsimd.tensor_relu(sb[:, B_Z:B_Z + L], zsub)
            zT = pool_tr.tile([P, L // P, P], FP8, tag="zT", name="zT")
            transpose4(nc, zT, sb[:, B_Z:B_Z + L], psA)

            # h_new / h2_new (finish the open accumulations)
            _mmdr(nc, hn_ps, zT, None, wzh, L // P // 2, first=False)
            nc.scalar.activation(sb[:, B_HN:B_HN + H], hn_ps, AF.Relu)
            _mmdr(nc, h2n_ps, zT, None, wzh2, L // P // 2, first=False)
            nc.scalar.activation(sb[:, B_H2N:B_H2N + H], h2n_ps, AF.Relu)

            # I_hat = sigmoid(z@W_rec.T - 2) = 0.5*tanh(0.5*x - 1) + 0.5
            for half in range(2):
                hsl = slice(half * 512, (half + 1) * 512)
                ih_ps = hn_ps if half == 0 else h2n_ps
                _mmdr(nc, ih_ps, zT, None, wrec, L // P // 2, n_slice=hsl)
                tsb = pool_im.tile([P, 512], F32, tag="scr2", bufs=5, name="tsb")
                nc.scalar.activation(tsb, ih_ps, AF.Tanh, scale=0.5, bias=neg1_col)
                ih_st = sb[:, B_IH + half * 512:B_IH + half * 512 + 512]
                nc.gpsimd.tensor_scalar(ih_st, tsb, 0.5, 0.5, OP.mult, OP.add)
                l1d = pool_im.tile([P, 512], F32, tag="scr3", bufs=4, name="l1d")
                nc.vector.scalar_tensor_tensor(
                    l1d, ih_st, -1.0, it16[:, hsl], OP.mult, OP.add
                )
                nc.gpsimd.tensor_mul(
                    sb[:, B_L1 + half * 512:B_L1 + half * 512 + 512], l1d, l1d
                )

            # layer_2_error = (z - mu_p - eps_zhat*sigma_p)^2
            e_sb = pool_im.tile([P, L], F32, tag="scr1", bufs=4, name="e_sb")
            nc.vector.tensor_mul(
                e_sb, pk[:, PB_EPSZH:PB_EPSZH + L], sa[:, A_SP:A_SP + L]
            )
            d_sb = pool_im.tile([P, L], F32, tag="scr2", bufs=5, name="d_sb")
            nc.vector.tensor_sub(d_sb, sb[:, B_Z:B_Z + L], st["mup"])
            d2 = pool_im.tile([P, L], F32, tag="scr3", bufs=4, name="d2")
            nc.vector.scalar_tensor_tensor(d2, e_sb, -1.0, d_sb, OP.mult, OP.add)
            nc.scalar.activation(sb[:, B_L2:B_L2 + L], d2, AF.Square)

            # output DMAs (contiguous blocks)
            nc.sync.dma_start(out=out_d[rows, OFF_SP:OFF_SP + 2 * L],
                              in_=sa[:, A_SP:A_SP + 2 * L])
            nc.sync.dma_start(out=out_d[rows, OFF_TFF:OFF_TFF + L],
                              in_=sa[:, A_TFF:A_TFF + L])
            nc.sync.dma_start(out=out_d[rows, OFF_Z:OFF_Z + 3 * L],
                              in_=sb[:, B_Z:B_Z + 3 * L])
            nc.sync.dma_start(out=out_d[rows, OFF_SST:OFF_SST + L],
                              in_=sb[:, B_SST:B_SST + L])
            nc.sync.dma_start(out=out_d[rows, OFF_ZE:OFF_ZE + L],
                              in_=sb[:, B_Z:B_Z + L])
            nc.sync.dma_start(out=out_d[rows, OFF_IH:OFF_IH + 2 * D + L],
                              in_=sb[:, B_IH:B_IH + 2 * D + L])

        states = {}
        for t in range(nt):
            pkb[t] = load_pack(pool_inb, packb_d, PB_W, t, "pkb")
            states[t] = stage1(t, pka.pop(t))
            if t >= DEPTH:
                tail(t - DEPTH, states.pop(t - DEPTH), pkb.pop(t - DEPTH))
            if t + 2 < nt and (t + 2) not in pka:
                pka[t + 2] = load_pack(pool_ina, packa_d, PA_W, t + 2, "pka")
        for t in range(nt - DEPTH, nt):
            tail(t, states.pop(t), pkb.pop(t))

    nc.compile()
    return nc


_NC_CACHE = []


def _get_program():
    if not _NC_CACHE:
        _NC_CACHE.append(_build_program())
    return _NC_CACHE[0]


def _prep_in_maps(inputs):
    f32 = lambda a: np.asarray(a, dtype=np.float32)
    it = f32(inputs["I_t"]).reshape(N_CORES, BL, D)
    packa = np.concatenate(
        [
            f32(inputs["sigma_p_prev"]).reshape(N_CORES, BL, L),
            f32(inputs["theta_ff_prev"]).reshape(N_CORES, BL, L),
            f32(inputs["theta_prev"]).reshape(N_CORES, BL, L),
            f32(inputs["eps_z"]).reshape(N_CORES, BL, L),
        ],
        axis=2,
    ).astype(NP_BF16)
    packb = np.concatenate(
        [
            it,
            f32(inputs["sst_inh_prev"]).reshape(N_CORES, BL, L),
            f32(inputs["eps_zhat"]).reshape(N_CORES, BL, L),
        ],
        axis=2,
    ).astype(NP_BF16)
    itT = np.ascontiguousarray(it.transpose(0, 2, 1)).astype(NP_FP8)
    hT = np.ascontiguousarray(
        f32(inputs["h"]).reshape(N_CORES, BL, H).transpose(0, 2, 1)
    ).astype(NP_BF16)
    h2T = np.ascontiguousarray(
        f32(inputs["h2"]).reshape(N_CORES, BL, H).transpose(0, 2, 1)
    ).astype(NP_BF16)

    trc = lambda a, d: np.ascontiguousarray(f32(a).T).astype(d)
    rep = {
        "wpm_t": trc(inputs["W_post_mu"], NP_FP8),
        "wps_t": trc(inputs["W_post_sigma"], NP_FP8),
        "wi2t_t": trc(inputs["W_I_to_theta"], NP_FP8),
        "wzh_t": trc(inputs["W_z_to_h"], NP_FP8),
        "wzh2_t": trc(inputs["W_z_to_h2"], NP_FP8),
        "wvip_t": trc(inputs["W_vip"], NP_FP8),
        "wt2z_t": trc(inputs["W_theta_to_z"], NP_FP8),
        "whh_t": trc(inputs["W_h_to_h"], NP_BF16),
        "wh2h2_t": trc(inputs["W_h2_to_h2"], NP_BF16),
        "wprs_t": trc(inputs["W_prior_sigma"], NP_BF16),
        "wprm_t": trc(inputs["W_prior_mu"], NP_BF16),
        "wrec1": np.ascontiguousarray(f32(inputs["W_rec1"])).astype(NP_BF16),
        "wrec2_t": trc(inputs["W_rec2"], NP_BF16),
        "bps": np.ascontiguousarray(f32(inputs["b_prior_sigma"]).reshape(1, L)),
    }
    return [
        {"packa": packa[i], "packb": packb[i], "itT": itT[i], "hT": hT[i],
         "h2T": h2T[i], **rep}
        for i in range(N_CORES)
    ]


def run(inputs, trace=False, **kw):
    nc = _get_program()
    in_maps = _prep_in_maps(inputs)
    res = run_bass_kernel_spmd(
        nc, in_maps, core_ids=list(range(N_CORES)), trace=trace, **kw
    )
    out = np.concatenate([res.results[i]["out"] for i in range(N_CORES)], axis=0)
    return out, res


def kernel(**inputs):
    out, _ = run(inputs)
    return out


# revision 12
# speedup vs baseline: 1.2429x; 1.0124x over previous
"""Trainium2 Bass kernel for EnergyConstrainedPredictiveCodingModel.

Data-parallel over the batch dim across 8 NeuronCores; weights replicated.

v3 design (see git history for the baseline):
  - DMA bytes 64MB -> ~44MB/core: activations bf16, split into an S1-pack
    [BL,2048] (spp|tffp|tp|epsz) and a tail-pack [BL,2048] (I_t|sstp|epszh);
    I_t/h/h2 additionally host-transposed (fp8/bf16/bf16); weights fp8/bf16.
  - PE: fp8e4m3 DoubleRow matmuls for ith/muq/sq/vip/sst/zh/zh2/recon; bf16
    for sigp/mup (l2err-critical) and hh/h2h2.  On-chip transposes only for
    sigma_p/theta/z (f32 in, fp8 cast at the PSUM evict).
  - Software pipeline depth 3 (S1(t) ... tail(t-3)) so every engine's
    in-order queue only sees ready instructions; independent matmuls are
    emitted before dependent transposes inside each stage.
  - Elementwise split across ACT (tanh/exp/abs/relu/copy: one table set),
    DVE (stt chains, fast approx reciprocal), Pool/GpSimd (SBUF-only ops).
  - Outputs staged in stgA [128,1536] (sigma_p|theta|theta_ff, from S1) and
    stgB [128,4608] (z|hn|h2n|sst|ihat|l1|l2, from tail); 6 contiguous
    output DMAs per row-tile (z_energy = 2nd DMA of the z column).
"""

import numpy as np
import ml_dtypes
from contextlib import ExitStack

import concourse.bass as bass
import concourse.mybir as mybir
import concourse.tile as tile
from concourse import bacc
from concourse.bass_utils import run_bass_kernel_spmd
from concourse.masks import make_identity

B, D, L, H, REC = 8192, 1024, 512, 512, 256
MAX_NORM = 0.5
N_CORES = 8
BL = B // N_CORES            # rows per core
P = 128                      # partitions
NT = BL // P                 # row tiles per core
OUT_W = 9 * L + 2 * D        # 6656
DEPTH = 3                    # software pipeline depth

F32 = mybir.dt.float32
F32R = mybir.dt.float32r
BF16 = mybir.dt.bfloat16
FP8 = mybir.dt.float8e4
AF = mybir.ActivationFunctionType
OP = mybir.AluOpType
DR = mybir.MatmulPerfMode.DoubleRow

NP_BF16 = ml_dtypes.bfloat16
NP_FP8 = ml_dtypes.float8_e4m3

# output column offsets
OFF_Z = 0
OFF_HN = L
OFF_H2N = 2 * L
OFF_SP = 3 * L
OFF_TH = 4 * L
OFF_SST = 5 * L
OFF_TFF = 6 * L
OFF_ZE = 7 * L
OFF_IH = 8 * L
OFF_L1 = 8 * L + D
OFF_L2 = 8 * L + 2 * D

# stgA columns: [sigma_p | theta | theta_ff]
A_SP, A_TH, A_TFF, A_W = 0, L, 2 * L, 3 * L
# stgB columns: [z | hn | h2n | sst | ihat | l1 | l2]
B_Z, B_HN, B_H2N, B_SST = 0, L, 2 * L, 3 * L
B_IH, B_L1, B_L2 = 4 * L, 4 * L + D, 4 * L + 2 * D
B_W = 4 * L + 2 * D + L      # 4608

# packA columns (S1-only inputs)
PA_SPP, PA_TFFP, PA_TP, PA_EPSZ, PA_W = 0, L, 2 * L, 3 * L, 4 * L
# packB columns (tail-only inputs)
PB_IT, PB_SSTP, PB_EPSZH, PB_W = 0, D, D + L, D + 2 * L


def _mm16(nc, out_ps, lhsT_sb, rows, w_sb, nk, first=True, last=True):
    for c in range(nk):
        nc.tensor.matmul(
            out_ps,
            lhsT_sb[:, c, rows],
            w_sb[:, c, :],
            start=(first and c == 0),
            stop=(last and c == nk - 1),
        )


def _mmdr(nc, out_ps, lhsT_sb, rows, w_sb, npair, first=True, last=True,
          n_slice=None):
    for c in range(npair):
        rhs = (w_sb[:, 2 * c:2 * c + 2, :] if n_slice is None
               else w_sb[:, 2 * c:2 * c + 2, n_slice])
        if rows is None:
            lhs = lhsT_sb[:, 2 * c:2 * c + 2, :]
        else:
            lhs = lhsT_sb[:, 2 * c:2 * c + 2, rows]
        nc.tensor.matmul(
            out_ps, lhs, rhs,
            start=(first and c == 0),
            stop=(last and c == npair - 1),
            perf_mode=DR,
        )


def _build_program(bl=BL):
    nc = bacc.Bacc(trn_type="TRN2", target_bir_lowering=False, debug=False)
    nt = bl // P

    def din(name, shape, dtype):
        return nc.dram_tensor(name, shape, dtype, kind="ExternalInput").ap()

    packa_d = din("packa", [bl, PA_W], BF16)
    packb_d = din("packb", [bl, PB_W], BF16)
    itT_d = din("itT", [D, bl], FP8)
    hT_d = din("hT", [H, bl], BF16)
    h2T_d = din("h2T", [H, bl], BF16)
    wpm_d = din("wpm_t", [D, L], FP8)
    wps_d = din("wps_t", [D, L], FP8)
    wi2t_d = din("wi2t_t", [D, L], FP8)
    wzh_d = din("wzh_t", [L, H], FP8)
    wzh2_d = din("wzh2_t", [L, H], FP8)
    wvip_d = din("wvip_t", [L, L], FP8)
    wt2z_d = din("wt2z_t", [L, L], FP8)
    whh_d = din("whh_t", [H, H], BF16)
    wh2h2_d = din("wh2h2_t", [H, H], BF16)
    wprs_d = din("wprs_t", [H, L], BF16)
    wprm_d = din("wprm_t", [H, L], BF16)
    wrec1_d = din("wrec1", [REC, L], BF16)
    wrec2_d = din("wrec2_t", [REC, D], BF16)
    bps_d = din("bps", [1, L], F32)

    out_d = nc.dram_tensor("out", [bl, OUT_W], F32, kind="ExternalOutput").ap()

    with tile.TileContext(nc) as tc, ExitStack() as ctx:
        weights = ctx.enter_context(tc.tile_pool(name="weights", bufs=1))
        consts = ctx.enter_context(tc.tile_pool(name="consts", bufs=1))
        psum = ctx.enter_context(tc.tile_pool(name="psum", bufs=8, space="PSUM"))
        pool_ina = ctx.enter_context(tc.tile_pool(name="inpa", bufs=3))
        pool_tin = ctx.enter_context(tc.tile_pool(name="tin", bufs=1))

        ident = consts.tile([P, P], F32)
        make_identity(nc, ident)
        ones_row = consts.tile([1, P], F32R)
        onesf = consts.tile([1, P], F32)
        nc.vector.memset(onesf, 1.0)
        nc.scalar.copy(ones_row, onesf)
        ones_col = consts.tile([P, 1], F32)
        nc.vector.memset(ones_col, 1.0)
        neg1_col = consts.tile([P, 1], F32)
        nc.vector.memset(neg1_col, -1.0)
        bps = consts.tile([1, L], F32R)

        # ---- big up-front input DMAs (transposed activations) ----
        itT = pool_tin.tile([P, D // P, bl], FP8, tag="itT")
        nc.sync.dma_start(out=itT, in_=itT_d.rearrange("(c p) n -> p c n", p=P))
        hT = pool_tin.tile([P, H // P, bl], BF16, tag="hT")
        nc.sync.dma_start(out=hT, in_=hT_d.rearrange("(c p) n -> p c n", p=P))
        h2T = pool_tin.tile([P, H // P, bl], BF16, tag="h2T")
        nc.sync.dma_start(out=h2T, in_=h2T_d.rearrange("(c p) n -> p c n", p=P))

        def load_pack(pool, dram, width, t, name):
            rows = slice(t * P, (t + 1) * P)
            pk = pool.tile([P, width], BF16, tag=name, name=name)
            nc.sync.dma_start(out=pk, in_=dram[rows, :])
            return pk

        pka = {t: load_pack(pool_ina, packa_d, PA_W, t, "pka") for t in range(2)}
        pkb = {}

        def wload(dram_ap, K, N, name, dtype):
            t = weights.tile([P, K // P, N], dtype, tag=name, name=name)
            nc.sync.dma_start(out=t, in_=dram_ap.rearrange("(c p) n -> p c n", p=P))
            return t

        wprs = wload(wprs_d, H, L, "wprs", BF16)
        bps_st = consts.tile([1, L], F32)
        nc.sync.dma_start(out=bps_st, in_=bps_d)
        nc.scalar.activation(bps, bps_st, AF.Relu)
        wi2t = wload(wi2t_d, D, L, "wi2t", FP8)
        wvip = wload(wvip_d, L, L, "wvip", FP8)
        nc.vector.tensor_scalar_max(
            wvip.rearrange("p c n -> p (c n)"), wvip.rearrange("p c n -> p (c n)"), 0.0
        )
        wprm = wload(wprm_d, H, L, "wprm", BF16)
        wpm = wload(wpm_d, D, L, "wpm", FP8)
        wps = wload(wps_d, D, L, "wps", FP8)
        wt2z = wload(wt2z_d, L, L, "wt2z", FP8)
        nc.vector.tensor_scalar_max(
            wt2z.rearrange("p c n -> p (c n)"), wt2z.rearrange("p c n -> p (c n)"), 0.0
        )
        wzh = wload(wzh_d, L, H, "wzh", FP8)
        wzh2 = wload(wzh2_d, L, H, "wzh2", FP8)
        wh2h2 = wload(wh2h2_d, H, H, "wh2h2", BF16)
        whh = weights.tile([P, H // P, H], BF16, tag="whh")
        wrec = weights.tile([P, L // P, D], FP8, tag="wrec")

        with tc.tile_pool(name="setup", bufs=1) as setup:
            # W_h_to_h spectral clip: W * min(1, MAX_NORM / ||W||_F)
            # (loaded straight into the final tile; scaled in place)
            nc.sync.dma_start(
                out=whh, in_=whh_d.rearrange("(c p) n -> p c n", p=P)
            )
            whh_f = whh.rearrange("p c n -> p (c n)")
            nchk = (H // P) * H // 512
            acc = setup.tile([P, nchk], F32)
            for j in range(nchk):
                scr = setup.tile([P, 512], F32, tag="scr")
                nc.scalar.activation(
                    scr, whh_f[:, j * 512:(j + 1) * 512], AF.Square,
                    accum_out=acc[:, j:j + 1],
                )
            sq_sum = setup.tile([P, 1], F32)
            nc.vector.tensor_reduce(sq_sum, acc, mybir.AxisListType.X, OP.add)
            nrm2_ps = psum.tile([1, 1], F32, tag="s1ps", bufs=3, name="nrm2_ps")
            nc.tensor.matmul(nrm2_ps, sq_sum, ones_col, start=True, stop=True)
            nrm = setup.tile([1, 1], F32)
            nc.scalar.activation(nrm, nrm2_ps, AF.Sqrt)
            rn = setup.tile([1, 1], F32)
            nc.vector.reciprocal(rn, nrm)
            scale = setup.tile([1, 1], F32)
            nc.vector.tensor_scalar(scale, rn, MAX_NORM, 1.0, OP.mult, OP.min)
            scale_ps = psum.tile([P, 1], F32, tag="s1ps", bufs=3, name="scale_ps")
            nc.tensor.matmul(scale_ps, onesf, scale, start=True, stop=True)
            scale_bc = setup.tile([P, 1], F32)
            nc.scalar.copy(scale_bc, scale_ps)
            nc.vector.tensor_scalar(whh_f, whh_f, scale_bc, None, OP.mult)

            # fuse W_rec = (W_rec2 @ W_rec1).T = W_rec1.T @ W_rec2.T (bf16)
            wrec1 = setup.tile([P, REC // P, L], BF16, tag="wrec1")
            nc.sync.dma_start(
                out=wrec1, in_=wrec1_d.rearrange("(c p) n -> p c n", p=P)
            )
            wrec2 = setup.tile([P, REC // P, D], BF16, tag="wrec2")
            nc.sync.dma_start(
                out=wrec2, in_=wrec2_d.rearrange("(c p) n -> p c n", p=P)
            )
            for m in range(L // P):
                for half in range(2):
                    ps = psum.tile([P, 512], F32, tag="tlps", bufs=3)
                    for c in range(REC // P):
                        nc.tensor.matmul(
                            ps,
                            wrec1[:, c, m * P:(m + 1) * P],
                            wrec2[:, c, half * 512:(half + 1) * 512],
                            start=(c == 0),
                            stop=(c == REC // P - 1),
                        )
                    nc.scalar.copy(wrec[:, m, half * 512:(half + 1) * 512], ps)

        pool_inb = ctx.enter_context(tc.tile_pool(name="inpb", bufs=DEPTH + 1))
        pool_sa = ctx.enter_context(tc.tile_pool(name="stga", bufs=DEPTH + 1))
        pool_stb = ctx.enter_context(tc.tile_pool(name="stgb", bufs=2))
        pool_s1 = ctx.enter_context(tc.tile_pool(name="s1t", bufs=DEPTH + 1))
        pool_im = ctx.enter_context(tc.tile_pool(name="interm", bufs=2))
        pool_tr = ctx.enter_context(tc.tile_pool(name="trans", bufs=2))

        def transpose4(nc, dst8, src, ps):
            for j in range(4):
                nc.tensor.transpose(
                    ps[:, j * P:(j + 1) * P], src[:, j * P:(j + 1) * P], ident
                )
            nc.scalar.copy(dst8.rearrange("p c n -> p (c n)"), ps)

        # ---------------- software-pipelined main loop ----------------
        def stage1(t, pk):
            rows = slice(t * P, (t + 1) * P)
            st = {"pka": pk, "rows": rows}
            sa = pool_sa.tile([P, A_W], F32, tag="sa", name="sa")
            st["sa"] = sa

            # --- independent matmuls first (PE never head-blocks) ---
            # PSUM: 5 banks; sigp's bank is reused by vip, ith's by the
            # sigma_p transpose.
            sigp_ps = psum.tile([P, L], F32, tag="sigps", bufs=2, name="sigp_ps")
            nc.tensor.matmul(sigp_ps, ones_row, bps, start=True, stop=False)
            _mm16(nc, sigp_ps, hT, rows, wprs, H // P, first=False)
            ith_ps = psum.tile([P, L], F32, tag="s1ps", bufs=3, name="ith_ps")
            _mmdr(nc, ith_ps, itT, rows, wi2t, D // P // 2)
            mup_ps = psum.tile([P, L], F32, tag="s1ps", bufs=3, name="mup_ps")
            _mm16(nc, mup_ps, h2T, rows, wprm, H // P)
            muq_ps = psum.tile([P, L], F32, tag="s1ps", bufs=3, name="muq_ps")
            _mmdr(nc, muq_ps, itT, rows, wpm, D // P // 2)
            sq_ps = psum.tile([P, L], F32, tag="s1ps", bufs=3, name="sq_ps")
            _mmdr(nc, sq_ps, itT, rows, wps, D // P // 2)

            # sigma_p = 0.8*relu(h@Wprs.T + b) + 0.2*spp
            tmp_sp = pool_im.tile([P, L], F32, tag="scr1", bufs=4, name="tmp_sp")
            nc.scalar.activation(tmp_sp, sigp_ps, AF.Relu, scale=0.8)
            nc.vector.scalar_tensor_tensor(
                sa[:, A_SP:A_SP + L], pk[:, PA_SPP:PA_SPP + L], 0.2, tmp_sp,
                OP.mult, OP.add,
            )

            # theta_ff = tanh(0.4*tffp + exp(-50|tffp|)*(I@Wi2t.T))^2
            a1 = pool_im.tile([P, L], F32, tag="scr2", bufs=5, name="a1")
            nc.scalar.activation(a1, pk[:, PA_TFFP:PA_TFFP + L], AF.Abs)
            nc.scalar.activation(a1, a1, AF.Exp, scale=-50.0)
            tffm = pool_im.tile([P, L], F32, tag="scr3", bufs=4, name="tffm")
            nc.vector.tensor_mul(tffm, a1, ith_ps)
            nc.vector.scalar_tensor_tensor(
                tffm, pk[:, PA_TFFP:PA_TFFP + L], 0.4, tffm, OP.mult, OP.add
            )
            nc.scalar.activation(tffm, tffm, AF.Tanh)
            nc.scalar.activation(sa[:, A_TFF:A_TFF + L], tffm, AF.Square)

            # mu_p (held in SBUF for the tail's l2err)
            mup_sb = pool_s1.tile([P, L], F32, tag="mup", name="mup_sb")
            nc.scalar.activation(mup_sb, mup_ps, AF.Relu)
            st["mup"] = mup_sb

            # raw_z = tanh(relu(muq) + eps_z*0.5*tanh(0.005*sq))
            s_sb = pool_im.tile([P, L], F32, tag="scr1", bufs=4, name="s_sb")
            nc.scalar.activation(s_sb, sq_ps, AF.Tanh, scale=0.005)
            rz = pool_s1.tile([P, L], F32, tag="rz", name="rz")
            nc.vector.scalar_tensor_tensor(
                rz, s_sb, 0.5, pk[:, PA_EPSZ:PA_EPSZ + L], OP.mult, OP.mult
            )
            nc.vector.scalar_tensor_tensor(
                rz, muq_ps, 0.0, rz, OP.max, OP.add
            )
            nc.scalar.activation(rz, rz, AF.Tanh)
            st["rz"] = rz

            # --- dependent: sigma_p transpose + vip matmul + theta ---
            # (transpose reuses ith's bank, vip reuses sigp's bank)
            spT = pool_tr.tile([P, L // P, P], FP8, tag="spT", bufs=3,
                               name="spT")
            spT_ps = psum.tile([P, L], F32, tag="s1ps", bufs=3, name="spT_ps")
            transpose4(nc, spT, sa[:, A_SP:A_SP + L], spT_ps)
            _mmdr(nc, sigp_ps, spT, None, wvip, L // P // 2)

            # theta = 0.1*tp + tff/(1 + vip)
            th = pool_im.tile([P, L], F32, tag="scr2", bufs=5, name="th")
            nc.vector.tensor_scalar_add(th, sigp_ps, 1.0)
            nc.vector.reciprocal_approx_fast(out=th, in_=th)
            nc.vector.scalar_tensor_tensor(
                th, sa[:, A_TFF:A_TFF + L], 1.0, th, OP.mult, OP.mult
            )
            nc.vector.scalar_tensor_tensor(
                sa[:, A_TH:A_TH + L], pk[:, PA_TP:PA_TP + L], 0.1, th,
                OP.mult, OP.add,
            )
            return st

        def tail(t, st, pk):
            rows = st["rows"]
            sa = st["sa"]
            sb = pool_stb.tile([P, B_W], F32, tag="sb", name="sb")
            it16 = pk[:, PB_IT:PB_IT + D]

            # PSUM: 3 banks; bank A: thT -> sst -> zT; bank B: hn -> ih0;
            # bank C: h2n -> ih1.
            psA = psum.tile([P, L], F32, tag="tlps", bufs=3, name="psA")
            thT = pool_tr.tile([P, L // P, P], FP8, tag="thT", bufs=3, name="thT")
            transpose4(nc, thT, sa[:, A_TH:A_TH + L], psA)
            hn_ps = psum.tile([P, H], F32, tag="tlps", bufs=3, name="hn_ps")
            _mm16(nc, hn_ps, hT, rows, whh, H // P, last=False)
            h2n_ps = psum.tile([P, H], F32, tag="tlps", bufs=3, name="h2n_ps")
            _mm16(nc, h2n_ps, h2T, rows, wh2h2, H // P, last=False)

            # sst_inh = 0.8*sstp + theta@Wt2z_p.T   (bank A again)
            _mmdr(nc, psA, thT, None, wt2z, L // P // 2)
            sst_st = sb[:, B_SST:B_SST + L]
            nc.vector.scalar_tensor_tensor(
                sst_st, pk[:, PB_SSTP:PB_SSTP + L], 0.8, psA, OP.mult, OP.add
            )

            # z = relu(raw_z - sst)
            zsub = pool_im.tile([P, L], F32, tag="scr1", bufs=4, name="zsub")
            nc.vector.scalar_tensor_tensor(
                zsub, sst_st, -1.0, st["rz"], OP.mult, OP.add
            )
            nc.gpsimd.tensor_relu(sb[:, B_Z:B_Z + L], zsub)
            zT = pool_tr.tile([P, L // P, P], FP8, tag="zT", bufs=3, name="zT")
            transpose4(nc, zT, sb[:, B_Z:B_Z + L], psA)

            # h_new / h2_new (finish the open accumulations)
            _mmdr(nc, hn_ps, zT, None, wzh, L // P // 2, first=False)
            nc.scalar.activation(sb[:, B_HN:B_HN + H], hn_ps, AF.Relu)
            _mmdr(nc, h2n_ps, zT, None, wzh2, L // P // 2, first=False)
            nc.scalar.activation(sb[:, B_H2N:B_H2N + H], h2n_ps, AF.Relu)

            # I_hat = sigmoid(z@W_rec.T - 2) = 0.5*tanh(0.5*x - 1) + 0.5
            for half in range(2):
                hsl = slice(half * 512, (half + 1) * 512)
                ih_ps = hn_ps if half == 0 else h2n_ps
                _mmdr(nc, ih_ps, zT, None, wrec, L // P // 2, n_slice=hsl)
                tsb = pool_im.tile([P, 512], F32, tag="scr2", bufs=5, name="tsb")
                nc.scalar.activation(tsb, ih_ps, AF.Tanh, scale=0.5, bias=neg1_col)
                ih_st = sb[:, B_IH + half * 512:B_IH + half * 512 + 512]
                nc.gpsimd.tensor_scalar(ih_st, tsb, 0.5, 0.5, OP.mult, OP.add)
                l1d = pool_im.tile([P, 512], F32, tag="scr3", bufs=4, name="l1d")
                nc.vector.scalar_tensor_tensor(
                    l1d, ih_st, -1.0, it16[:, hsl], OP.mult, OP.add
                )
                nc.gpsimd.tensor_mul(
                    sb[:, B_L1 + half * 512:B_L1 + half * 512 + 512], l1d, l1d
                )

            # layer_2_error = (z - mu_p - eps_zhat*sigma_p)^2
            e_sb = pool_im.tile([P, L], F32, tag="scr1", bufs=4, name="e_sb")
            nc.vector.tensor_mul(
                e_sb, pk[:, PB_EPSZH:PB_EPSZH + L], sa[:, A_SP:A_SP + L]
            )
            d_sb = pool_im.tile([P, L], F32, tag="scr2", bufs=5, name="d_sb")
            nc.vector.tensor_sub(d_sb, sb[:, B_Z:B_Z + L], st["mup"])
            d2 = pool_im.tile([P, L], F32, tag="scr3", bufs=4, name="d2")
            nc.vector.scalar_tensor_tensor(d2, e_sb, -1.0, d_sb, OP.mult, OP.add)
            nc.scalar.activation(sb[:, B_L2:B_L2 + L], d2, AF.Square)

            # output DMAs (contiguous blocks)
            nc.sync.dma_start(out=out_d[rows, OFF_SP:OFF_SP + 2 * L],
                              in_=sa[:, A_SP:A_SP + 2 * L])
            nc.sync.dma_start(out=out_d[rows, OFF_TFF:OFF_TFF + L],
                              in_=sa[:, A_TFF:A_TFF + L])
            nc.sync.dma_start(out=out_d[rows, OFF_Z:OFF_Z + 3 * L],
                              in_=sb[:, B_Z:B_Z + 3 * L])
            nc.sync.dma_start(out=out_d[rows, OFF_SST:OFF_SST + L],
                              in_=sb[:, B_SST:B_SST + L])
            nc.sync.dma_start(out=out_d[rows, OFF_ZE:OFF_ZE + L],
                              in_=sb[:, B_Z:B_Z + L])
            nc.sync.dma_start(out=out_d[rows, OFF_IH:OFF_IH + 2 * D + L],
                              in_=sb[:, B_IH:B_IH + 2 * D + L])

        states = {}
        for t in range(nt):
            pkb[t] = load_pack(pool_inb, packb_d, PB_W, t, "pkb")
            states[t] = stage1(t, pka.pop(t))
            if t >= DEPTH:
                tail(t - DEPTH, states.pop(t - DEPTH), pkb.pop(t - DEPTH))
            if t + 2 < nt and (t + 2) not in pka:
                pka[t + 2] = load_pack(pool_ina, packa_d, PA_W, t + 2, "pka")
        for t in range(nt - DEPTH, nt):
            tail(t, states.pop(t), pkb.pop(t))

    nc.compile()
    return nc


_NC_CACHE = []


def _get_program():
    if not _NC_CACHE:
        _NC_CACHE.append(_build_program())
    return _NC_CACHE[0]


def _prep_in_maps(inputs):
    f32 = lambda a: np.asarray(a, dtype=np.float32)
    it = f32(inputs["I_t"]).reshape(N_CORES, BL, D)
    packa = np.concatenate(
        [
            f32(inputs["sigma_p_prev"]).reshape(N_CORES, BL, L),
            f32(inputs["theta_ff_prev"]).reshape(N_CORES, BL, L),
            f32(inputs["theta_prev"]).reshape(N_CORES, BL, L),
            f32(inputs["eps_z"]).reshape(N_CORES, BL, L),
        ],
        axis=2,
    ).astype(NP_BF16)
    packb = np.concatenate(
        [
            it,
            f32(inputs["sst_inh_prev"]).reshape(N_CORES, BL, L),
            f32(inputs["eps_zhat"]).reshape(N_CORES, BL, L),
        ],
        axis=2,
    ).astype(NP_BF16)
    itT = np.ascontiguousarray(it.transpose(0, 2, 1)).astype(NP_FP8)
    hT = np.ascontiguousarray(
        f32(inputs["h"]).reshape(N_CORES, BL, H).transpose(0, 2, 1)
    ).astype(NP_BF16)
    h2T = np.ascontiguousarray(
        f32(inputs["h2"]).reshape(N_CORES, BL, H).transpose(0, 2, 1)
    ).astype(NP_BF16)

    trc = lambda a, d: np.ascontiguousarray(f32(a).T).astype(d)
    rep = {
        "wpm_t": trc(inputs["W_post_mu"], NP_FP8),
        "wps_t": trc(inputs["W_post_sigma"], NP_FP8),
        "wi2t_t": trc(inputs["W_I_to_theta"], NP_FP8),
        "wzh_t": trc(inputs["W_z_to_h"], NP_FP8),
        "wzh2_t": trc(inputs["W_z_to_h2"], NP_FP8),
        "wvip_t": trc(inputs["W_vip"], NP_FP8),
        "wt2z_t": trc(inputs["W_theta_to_z"], NP_FP8),
        "whh_t": trc(inputs["W_h_to_h"], NP_BF16),
        "wh2h2_t": trc(inputs["W_h2_to_h2"], NP_BF16),
        "wprs_t": trc(inputs["W_prior_sigma"], NP_BF16),
        "wprm_t": trc(inputs["W_prior_mu"], NP_BF16),
        "wrec1": np.ascontiguousarray(f32(inputs["W_rec1"])).astype(NP_BF16),
        "wrec2_t": trc(inputs["W_rec2"], NP_BF16),
        "bps": np.ascontiguousarray(f32(inputs["b_prior_sigma"]).reshape(1, L)),
    }
    return [
        {"packa": packa[i], "packb": packb[i], "itT": itT[i], "hT": hT[i],
         "h2T": h2T[i], **rep}
        for i in range(N_CORES)
    ]


def run(inputs, trace=False, **kw):
    nc = _get_program()
    in_maps = _prep_in_maps(inputs)
    res = run_bass_kernel_spmd(
        nc, in_maps, core_ids=list(range(N_CORES)), trace=trace, **kw
    )
    out = np.concatenate([res.results[i]["out"] for i in range(N_CORES)], axis=0)
    return out, res


def kernel(**inputs):
    out, _ = run(inputs)
    return out


# revision 13
# speedup vs baseline: 1.5885x; 1.2780x over previous
"""Trainium2 Bass kernel for EnergyConstrainedPredictiveCodingModel.

Data-parallel over the batch dim across 8 NeuronCores; weights replicated.

v3 design (see git history for the baseline):
  - DMA bytes 64MB -> ~44MB/core: activations bf16, split into an S1-pack
    [BL,2048] (spp|tffp|tp|epsz) and a tail-pack [BL,2048] (I_t|sstp|epszh);
    I_t/h/h2 additionally host-transposed (fp8/bf16/bf16); weights fp8/bf16.
  - PE: fp8e4m3 DoubleRow matmuls for ith/muq/sq/vip/sst/zh/zh2/recon; bf16
    for sigp/mup (l2err-critical) and hh/h2h2.  On-chip transposes only for
    sigma_p/theta/z (f32 in, fp8 cast at the PSUM evict).
  - Software pipeline depth 3 (S1(t) ... tail(t-3)) so every engine's
    in-order queue only sees ready instructions; independent matmuls are
    emitted before dependent transposes inside each stage.
  - Elementwise split across ACT (tanh/exp/abs/relu/copy: one table set),
    DVE (stt chains, fast approx reciprocal), Pool/GpSimd (SBUF-only ops).
  - Outputs staged in stgA [128,1536] (sigma_p|theta|theta_ff, from S1) and
    stgB [128,4608] (z|hn|h2n|sst|ihat|l1|l2, from tail); 6 contiguous
    output DMAs per row-tile (z_energy = 2nd DMA of the z column).
"""

import numpy as np
import ml_dtypes
from contextlib import ExitStack

import concourse.bass as bass
import concourse.mybir as mybir
import concourse.tile as tile
from concourse import bacc
from concourse.bass_utils import run_bass_kernel_spmd
from concourse.masks import make_identity

B, D, L, H, REC = 8192, 1024, 512, 512, 256
MAX_NORM = 0.5
N_CORES = 8
BL = B // N_CORES            # rows per core
P = 128                      # partitions
NT = BL // P                 # row tiles per core
OUT_W = 9 * L + 2 * D        # 6656
DEPTH = 3                    # software pipeline depth

F32 = mybir.dt.float32
F32R = mybir.dt.float32r
BF16 = mybir.dt.bfloat16
FP8 = mybir.dt.float8e4
AF = mybir.ActivationFunctionType
OP = mybir.AluOpType
DR = mybir.MatmulPerfMode.DoubleRow

NP_BF16 = ml_dtypes.bfloat16
NP_FP8 = ml_dtypes.float8_e4m3

# output column offsets
OFF_Z = 0
OFF_HN = L
OFF_H2N = 2 * L
OFF_SP = 3 * L
OFF_TH = 4 * L
OFF_SST = 5 * L
OFF_TFF = 6 * L
OFF_ZE = 7 * L
OFF_IH = 8 * L
OFF_L1 = 8 * L + D
OFF_L2 = 8 * L + 2 * D

# stgA columns: [sigma_p | theta | theta_ff]
A_SP, A_TH, A_TFF, A_W = 0, L, 2 * L, 3 * L
# stgB1 columns: [z | hn | h2n | sst]; stgB2 columns: [ihat | l1 | l2]
B_Z, B_HN, B_H2N, B_SST, B1_W = 0, L, 2 * L, 3 * L, 4 * L
B_IH, B_L1, B_L2 = 0, D, 2 * D
B2_W = 2 * D + L             # 2560

# packA columns (S1-only inputs)
PA_SPP, PA_TFFP, PA_TP, PA_EPSZ, PA_W = 0, L, 2 * L, 3 * L, 4 * L
# packB columns (tail-only inputs)
PB_IT, PB_SSTP, PB_EPSZH, PB_W = 0, D, D + L, D + 2 * L


def _mm16(nc, out_ps, lhsT_sb, rows, w_sb, nk, first=True, last=True):
    for c in range(nk):
        nc.tensor.matmul(
            out_ps,
            lhsT_sb[:, c, rows],
            w_sb[:, c, :],
            start=(first and c == 0),
            stop=(last and c == nk - 1),
        )


def _mmdr(nc, out_ps, lhsT_sb, rows, w_sb, npair, first=True, last=True,
          n_slice=None):
    for c in range(npair):
        rhs = (w_sb[:, 2 * c:2 * c + 2, :] if n_slice is None
               else w_sb[:, 2 * c:2 * c + 2, n_slice])
        if rows is None:
            lhs = lhsT_sb[:, 2 * c:2 * c + 2, :]
        else:
            lhs = lhsT_sb[:, 2 * c:2 * c + 2, rows]
        nc.tensor.matmul(
            out_ps, lhs, rhs,
            start=(first and c == 0),
            stop=(last and c == npair - 1),
            perf_mode=DR,
        )


def _build_program(bl=BL):
    nc = bacc.Bacc(trn_type="TRN2", target_bir_lowering=False, debug=False)
    nt = bl // P

    def din(name, shape, dtype):
        return nc.dram_tensor(name, shape, dtype, kind="ExternalInput").ap()

    packa_d = din("packa", [bl, PA_W], BF16)
    packb_d = din("packb", [bl, PB_W], BF16)
    itT_d = din("itT", [D, bl], FP8)
    hT_d = din("hT", [H, bl], BF16)
    h2T_d = din("h2T", [H, bl], BF16)
    wpm_d = din("wpm_t", [D, L], FP8)
    wps_d = din("wps_t", [D, L], FP8)
    wi2t_d = din("wi2t_t", [D, L], FP8)
    wzh_d = din("wzh_t", [L, H], FP8)
    wzh2_d = din("wzh2_t", [L, H], FP8)
    wvip_d = din("wvip_t", [L, L], FP8)
    wt2z_d = din("wt2z_t", [L, L], FP8)
    whh_d = din("whh_t", [H, H], BF16)
    wh2h2_d = din("wh2h2_t", [H, H], BF16)
    wprs_d = din("wprs_t", [H, L], BF16)
    wprm_d = din("wprm_t", [H, L], BF16)
    wrec1_d = din("wrec1", [REC, L], BF16)
    wrec2_d = din("wrec2_t", [REC, D], BF16)
    bps_d = din("bps", [1, L], F32)

    out_d = nc.dram_tensor("out", [bl, OUT_W], F32, kind="ExternalOutput").ap()

    with tile.TileContext(nc) as tc, ExitStack() as ctx:
        weights = ctx.enter_context(tc.tile_pool(name="weights", bufs=1))
        consts = ctx.enter_context(tc.tile_pool(name="consts", bufs=1))
        psum = ctx.enter_context(tc.tile_pool(name="psum", bufs=8, space="PSUM"))
        pool_ina = ctx.enter_context(tc.tile_pool(name="inpa", bufs=3))
        pool_tin = ctx.enter_context(tc.tile_pool(name="tin", bufs=1))

        ident = consts.tile([P, P], F32)
        make_identity(nc, ident)
        ones_row = consts.tile([1, P], F32R)
        onesf = consts.tile([1, P], F32)
        nc.vector.memset(onesf, 1.0)
        nc.scalar.copy(ones_row, onesf)
        ones_col = consts.tile([P, 1], F32)
        nc.vector.memset(ones_col, 1.0)
        neg1_col = consts.tile([P, 1], F32)
        nc.vector.memset(neg1_col, -1.0)
        bps = consts.tile([1, L], F32R)

        # ---- big up-front input DMAs (transposed activations) ----
        itT = pool_tin.tile([P, D // P, bl], FP8, tag="itT")
        nc.sync.dma_start(out=itT, in_=itT_d.rearrange("(c p) n -> p c n", p=P))
        hT = pool_tin.tile([P, H // P, bl], BF16, tag="hT")
        nc.sync.dma_start(out=hT, in_=hT_d.rearrange("(c p) n -> p c n", p=P))
        h2T = pool_tin.tile([P, H // P, bl], BF16, tag="h2T")
        nc.sync.dma_start(out=h2T, in_=h2T_d.rearrange("(c p) n -> p c n", p=P))

        def load_pack(pool, dram, width, t, name):
            rows = slice(t * P, (t + 1) * P)
            pk = pool.tile([P, width], BF16, tag=name, name=name)
            nc.sync.dma_start(out=pk, in_=dram[rows, :])
            return pk

        pka = {t: load_pack(pool_ina, packa_d, PA_W, t, "pka") for t in range(2)}
        pkb = {}

        def wload(dram_ap, K, N, name, dtype):
            t = weights.tile([P, K // P, N], dtype, tag=name, name=name)
            nc.sync.dma_start(out=t, in_=dram_ap.rearrange("(c p) n -> p c n", p=P))
            return t

        wprs = wload(wprs_d, H, L, "wprs", BF16)
        bps_st = consts.tile([1, L], F32)
        nc.sync.dma_start(out=bps_st, in_=bps_d)
        nc.scalar.activation(bps, bps_st, AF.Relu)
        wi2t = wload(wi2t_d, D, L, "wi2t", FP8)
        wvip = wload(wvip_d, L, L, "wvip", FP8)
        nc.vector.tensor_scalar_max(
            wvip.rearrange("p c n -> p (c n)"), wvip.rearrange("p c n -> p (c n)"), 0.0
        )
        wprm = wload(wprm_d, H, L, "wprm", BF16)
        wpm = wload(wpm_d, D, L, "wpm", FP8)
        wps = wload(wps_d, D, L, "wps", FP8)
        wt2z = wload(wt2z_d, L, L, "wt2z", FP8)
        nc.vector.tensor_scalar_max(
            wt2z.rearrange("p c n -> p (c n)"), wt2z.rearrange("p c n -> p (c n)"), 0.0
        )
        wzh = wload(wzh_d, L, H, "wzh", FP8)
        wzh2 = wload(wzh2_d, L, H, "wzh2", FP8)
        wh2h2 = wload(wh2h2_d, H, H, "wh2h2", BF16)
        whh = weights.tile([P, H // P, H], BF16, tag="whh")
        wrec = weights.tile([P, L // P, D], FP8, tag="wrec")

        with tc.tile_pool(name="setup", bufs=1) as setup:
            # W_h_to_h spectral clip: W * min(1, MAX_NORM / ||W||_F)
            # (loaded straight into the final tile; scaled in place)
            nc.sync.dma_start(
                out=whh, in_=whh_d.rearrange("(c p) n -> p c n", p=P)
            )
            whh_f = whh.rearrange("p c n -> p (c n)")
            nchk = (H // P) * H // 512
            acc = setup.tile([P, nchk], F32)
            for j in range(nchk):
                scr = setup.tile([P, 512], F32, tag="scr")
                nc.scalar.activation(
                    scr, whh_f[:, j * 512:(j + 1) * 512], AF.Square,
                    accum_out=acc[:, j:j + 1],
                )
            sq_sum = setup.tile([P, 1], F32)
            nc.vector.tensor_reduce(sq_sum, acc, mybir.AxisListType.X, OP.add)
            nrm2_ps = psum.tile([1, 1], F32, tag="s1ps", bufs=3, name="nrm2_ps")
            nc.tensor.matmul(nrm2_ps, sq_sum, ones_col, start=True, stop=True)
            nrm = setup.tile([1, 1], F32)
            nc.scalar.activation(nrm, nrm2_ps, AF.Sqrt)
            rn = setup.tile([1, 1], F32)
            nc.vector.reciprocal(rn, nrm)
            scale = setup.tile([1, 1], F32)
            nc.vector.tensor_scalar(scale, rn, MAX_NORM, 1.0, OP.mult, OP.min)
            scale_ps = psum.tile([P, 1], F32, tag="s1ps", bufs=3, name="scale_ps")
            nc.tensor.matmul(scale_ps, onesf, scale, start=True, stop=True)
            scale_bc = setup.tile([P, 1], F32)
            nc.scalar.copy(scale_bc, scale_ps)
            nc.vector.tensor_scalar(whh_f, whh_f, scale_bc, None, OP.mult)

            # fuse W_rec = (W_rec2 @ W_rec1).T = W_rec1.T @ W_rec2.T (bf16)
            wrec1 = setup.tile([P, REC // P, L], BF16, tag="wrec1")
            nc.sync.dma_start(
                out=wrec1, in_=wrec1_d.rearrange("(c p) n -> p c n", p=P)
            )
            wrec2 = setup.tile([P, REC // P, D], BF16, tag="wrec2")
            nc.sync.dma_start(
                out=wrec2, in_=wrec2_d.rearrange("(c p) n -> p c n", p=P)
            )
            for m in range(L // P):
                for half in range(2):
                    ps = psum.tile([P, 512], F32, tag="tlps", bufs=3)
                    for c in range(REC // P):
                        nc.tensor.matmul(
                            ps,
                            wrec1[:, c, m * P:(m + 1) * P],
                            wrec2[:, c, half * 512:(half + 1) * 512],
                            start=(c == 0),
                            stop=(c == REC // P - 1),
                        )
                    nc.scalar.copy(wrec[:, m, half * 512:(half + 1) * 512], ps)

        pool_inb = ctx.enter_context(tc.tile_pool(name="inpb", bufs=DEPTH + 1))
        pool_sa = ctx.enter_context(tc.tile_pool(name="stga", bufs=DEPTH + 1))
        pool_stb = ctx.enter_context(tc.tile_pool(name="stgb", bufs=2))
        pool_s1 = ctx.enter_context(tc.tile_pool(name="s1t", bufs=DEPTH + 1))
        pool_im = ctx.enter_context(tc.tile_pool(name="interm", bufs=2))
        pool_tr = ctx.enter_context(tc.tile_pool(name="trans", bufs=2))

        def transpose4(nc, dst8, src, ps):
            for j in range(4):
                nc.tensor.transpose(
                    ps[:, j * P:(j + 1) * P], src[:, j * P:(j + 1) * P], ident
                )
            nc.scalar.copy(dst8.rearrange("p c n -> p (c n)"), ps)

        # ---------------- software-pipelined main loop ----------------
        def stage1(t, pk):
            rows = slice(t * P, (t + 1) * P)
            st = {"pka": pk, "rows": rows}
            sa = pool_sa.tile([P, A_W], F32, tag="sa", name="sa")
            st["sa"] = sa

            # --- independent matmuls first (PE never head-blocks) ---
            # PSUM: 5 banks; sigp's bank is reused by vip, ith's by the
            # sigma_p transpose.
            sigp_ps = psum.tile([P, L], F32, tag="sigps", bufs=2, name="sigp_ps")
            nc.tensor.matmul(sigp_ps, ones_row, bps, start=True, stop=False)
            _mm16(nc, sigp_ps, hT, rows, wprs, H // P, first=False)
            ith_ps = psum.tile([P, L], F32, tag="s1ps", bufs=3, name="ith_ps")
            _mmdr(nc, ith_ps, itT, rows, wi2t, D // P // 2)
            mup_ps = psum.tile([P, L], F32, tag="s1ps", bufs=3, name="mup_ps")
            _mm16(nc, mup_ps, h2T, rows, wprm, H // P)
            muq_ps = psum.tile([P, L], F32, tag="s1ps", bufs=3, name="muq_ps")
            _mmdr(nc, muq_ps, itT, rows, wpm, D // P // 2)
            sq_ps = psum.tile([P, L], F32, tag="s1ps", bufs=3, name="sq_ps")
            _mmdr(nc, sq_ps, itT, rows, wps, D // P // 2)

            # sigma_p = 0.8*relu(h@Wprs.T + b) + 0.2*spp
            tmp_sp = pool_im.tile([P, L], F32, tag="scr1", bufs=4, name="tmp_sp")
            nc.scalar.activation(tmp_sp, sigp_ps, AF.Relu, scale=0.8)
            nc.vector.scalar_tensor_tensor(
                sa[:, A_SP:A_SP + L], pk[:, PA_SPP:PA_SPP + L], 0.2, tmp_sp,
                OP.mult, OP.add,
            )

            # theta_ff = tanh(0.4*tffp + exp(-50|tffp|)*(I@Wi2t.T))^2
            a1 = pool_im.tile([P, L], F32, tag="scr2", bufs=5, name="a1")
            nc.scalar.activation(a1, pk[:, PA_TFFP:PA_TFFP + L], AF.Abs)
            nc.scalar.activation(a1, a1, AF.Exp, scale=-50.0)
            tffm = pool_im.tile([P, L], F32, tag="scr3", bufs=4, name="tffm")
            nc.vector.tensor_mul(tffm, a1, ith_ps)
            nc.vector.scalar_tensor_tensor(
                tffm, pk[:, PA_TFFP:PA_TFFP + L], 0.4, tffm, OP.mult, OP.add
            )
            nc.scalar.activation(tffm, tffm, AF.Tanh)
            nc.scalar.activation(sa[:, A_TFF:A_TFF + L], tffm, AF.Square)

            # mu_p (held in SBUF for the tail's l2err)
            mup_sb = pool_s1.tile([P, L], F32, tag="mup", name="mup_sb")
            nc.scalar.activation(mup_sb, mup_ps, AF.Relu)
            st["mup"] = mup_sb

            # raw_z = tanh(relu(muq) + eps_z*0.5*tanh(0.005*sq))
            s_sb = pool_im.tile([P, L], F32, tag="scr1", bufs=4, name="s_sb")
            nc.scalar.activation(s_sb, sq_ps, AF.Tanh, scale=0.005)
            rz = pool_s1.tile([P, L], F32, tag="rz", name="rz")
            nc.vector.scalar_tensor_tensor(
                rz, s_sb, 0.5, pk[:, PA_EPSZ:PA_EPSZ + L], OP.mult, OP.mult
            )
            nc.vector.scalar_tensor_tensor(
                rz, muq_ps, 0.0, rz, OP.max, OP.add
            )
            nc.scalar.activation(rz, rz, AF.Tanh)
            st["rz"] = rz

            # --- dependent: sigma_p transpose + vip matmul + theta ---
            # (transpose reuses ith's bank, vip reuses sigp's bank)
            spT = pool_tr.tile([P, L // P, P], FP8, tag="spT", bufs=3,
                               name="spT")
            spT_ps = psum.tile([P, L], F32, tag="s1ps", bufs=3, name="spT_ps")
            transpose4(nc, spT, sa[:, A_SP:A_SP + L], spT_ps)
            _mmdr(nc, sigp_ps, spT, None, wvip, L // P // 2)

            # theta = 0.1*tp + tff/(1 + vip)
            th = pool_im.tile([P, L], F32, tag="scr2", bufs=5, name="th")
            nc.vector.tensor_scalar_add(th, sigp_ps, 1.0)
            nc.vector.reciprocal_approx_fast(out=th, in_=th)
            nc.vector.scalar_tensor_tensor(
                th, sa[:, A_TFF:A_TFF + L], 1.0, th, OP.mult, OP.mult
            )
            nc.vector.scalar_tensor_tensor(
                sa[:, A_TH:A_TH + L], pk[:, PA_TP:PA_TP + L], 0.1, th,
                OP.mult, OP.add,
            )
            return st

        def tail(t, st, pk):
            rows = st["rows"]
            sa = st["sa"]
            sb1 = pool_stb.tile([P, B1_W], F32, tag="sb1", name="sb1")
            sb2 = pool_stb.tile([P, B2_W], F32, tag="sb2", name="sb2")
            it16 = pk[:, PB_IT:PB_IT + D]

            # PSUM: bank A: thT -> sst; bank B: h2h2 -> ih1;
            # bank C: zT -> hh+zh -> ih0.  (zT out of bank A so consecutive
            # tails only serialize through the short thT->sst chain.)
            psA = psum.tile([P, L], F32, tag="tlps", bufs=3, name="psA")
            thT = pool_tr.tile([P, L // P, P], FP8, tag="thT", bufs=3, name="thT")
            transpose4(nc, thT, sa[:, A_TH:A_TH + L], psA)
            h2n_ps = psum.tile([P, H], F32, tag="tlps", bufs=3, name="h2n_ps")
            _mm16(nc, h2n_ps, h2T, rows, wh2h2, H // P, last=False)

            # sst_inh = 0.8*sstp + theta@Wt2z_p.T   (bank A again)
            _mmdr(nc, psA, thT, None, wt2z, L // P // 2)
            sst_st = sb1[:, B_SST:B_SST + L]
            nc.vector.scalar_tensor_tensor(
                sst_st, pk[:, PB_SSTP:PB_SSTP + L], 0.8, psA, OP.mult, OP.add
            )

            # z = relu(raw_z - sst)  (both steps on DVE: shortest chain)
            zsub = pool_im.tile([P, L], F32, tag="scr1", bufs=4, name="zsub")
            nc.vector.scalar_tensor_tensor(
                zsub, sst_st, -1.0, st["rz"], OP.mult, OP.add
            )
            nc.vector.tensor_scalar_max(sb1[:, B_Z:B_Z + L], zsub, 0.0)
            hn_ps = psum.tile([P, H], F32, tag="tlps", bufs=3, name="hn_ps")
            zT = pool_tr.tile([P, L // P, P], FP8, tag="zT", bufs=3, name="zT")
            transpose4(nc, zT, sb1[:, B_Z:B_Z + L], hn_ps)

            # h_new / h2_new
            _mm16(nc, hn_ps, hT, rows, whh, H // P, last=False)
            _mmdr(nc, hn_ps, zT, None, wzh, L // P // 2, first=False)
            nc.scalar.activation(sb1[:, B_HN:B_HN + H], hn_ps, AF.Relu)
            _mmdr(nc, h2n_ps, zT, None, wzh2, L // P // 2, first=False)
            nc.scalar.activation(sb1[:, B_H2N:B_H2N + H], h2n_ps, AF.Relu)

            # I_hat = sigmoid(z@W_rec.T - 2) = 0.5*tanh(0.5*x - 1) + 0.5
            for half in range(2):
                hsl = slice(half * 512, (half + 1) * 512)
                ih_ps = hn_ps if half == 0 else h2n_ps
                _mmdr(nc, ih_ps, zT, None, wrec, L // P // 2, n_slice=hsl)
                tsb = pool_im.tile([P, 512], F32, tag="scr2", bufs=5, name="tsb")
                nc.scalar.activation(tsb, ih_ps, AF.Tanh, scale=0.5, bias=neg1_col)
                ih_st = sb2[:, B_IH + half * 512:B_IH + half * 512 + 512]
                nc.gpsimd.tensor_scalar(ih_st, tsb, 0.5, 0.5, OP.mult, OP.add)
                l1d = pool_im.tile([P, 512], F32, tag="scr3", bufs=4, name="l1d")
                nc.vector.scalar_tensor_tensor(
                    l1d, ih_st, -1.0, it16[:, hsl], OP.mult, OP.add
                )
                nc.gpsimd.tensor_mul(
                    sb2[:, B_L1 + half * 512:B_L1 + half * 512 + 512], l1d, l1d
                )

            # layer_2_error = (z - mu_p - eps_zhat*sigma_p)^2
            e_sb = pool_im.tile([P, L], F32, tag="scr1", bufs=4, name="e_sb")
            nc.vector.tensor_mul(
                e_sb, pk[:, PB_EPSZH:PB_EPSZH + L], sa[:, A_SP:A_SP + L]
            )
            d_sb = pool_im.tile([P, L], F32, tag="scr2", bufs=5, name="d_sb")
            nc.vector.tensor_sub(d_sb, sb1[:, B_Z:B_Z + L], st["mup"])
            d2 = pool_im.tile([P, L], F32, tag="scr3", bufs=4, name="d2")
            nc.vector.scalar_tensor_tensor(d2, e_sb, -1.0, d_sb, OP.mult, OP.add)
            nc.scalar.activation(sb2[:, B_L2:B_L2 + L], d2, AF.Square)

            # output DMAs (contiguous blocks; issue split across SP and the
            # scalar engine's sequencer to halve per-engine issue cost)
            nc.sync.dma_start(out=out_d[rows, OFF_SP:OFF_SP + 2 * L],
                              in_=sa[:, A_SP:A_SP + 2 * L])
            nc.scalar.dma_start(out=out_d[rows, OFF_TFF:OFF_TFF + L],
                                in_=sa[:, A_TFF:A_TFF + L])
            nc.sync.dma_start(out=out_d[rows, OFF_Z:OFF_Z + 3 * L],
                              in_=sb1[:, B_Z:B_Z + 3 * L])
            nc.scalar.dma_start(out=out_d[rows, OFF_SST:OFF_SST + L],
                                in_=sb1[:, B_SST:B_SST + L])
            nc.scalar.dma_start(out=out_d[rows, OFF_ZE:OFF_ZE + L],
                                in_=sb1[:, B_Z:B_Z + L])
            nc.sync.dma_start(out=out_d[rows, OFF_IH:OFF_IH + 2 * D + L],
                              in_=sb2[:, B_IH:B_IH + 2 * D + L])

        states = {}
        for t in range(nt):
            pkb[t] = load_pack(pool_inb, packb_d, PB_W, t, "pkb")
            states[t] = stage1(t, pka.pop(t))
            if t >= DEPTH:
                tail(t - DEPTH, states.pop(t - DEPTH), pkb.pop(t - DEPTH))
            if t + 2 < nt and (t + 2) not in pka:
                pka[t + 2] = load_pack(pool_ina, packa_d, PA_W, t + 2, "pka")
        for t in range(nt - DEPTH, nt):
            tail(t, states.pop(t), pkb.pop(t))

    nc.compile()
    return nc


_NC_CACHE = []


def _get_program():
    if not _NC_CACHE:
        _NC_CACHE.append(_build_program())
    return _NC_CACHE[0]


def _prep_in_maps(inputs):
    f32 = lambda a: np.asarray(a, dtype=np.float32)
    it = f32(inputs["I_t"]).reshape(N_CORES, BL, D)
    packa = np.concatenate(
        [
            f32(inputs["sigma_p_prev"]).reshape(N_CORES, BL, L),
            f32(inputs["theta_ff_prev"]).reshape(N_CORES, BL, L),
            f32(inputs["theta_prev"]).reshape(N_CORES, BL, L),
            f32(inputs["eps_z"]).reshape(N_CORES, BL, L),
        ],
        axis=2,
    ).astype(NP_BF16)
    packb = np.concatenate(
        [
            it,
            f32(inputs["sst_inh_prev"]).reshape(N_CORES, BL, L),
            f32(inputs["eps_zhat"]).reshape(N_CORES, BL, L),
        ],
        axis=2,
    ).astype(NP_BF16)
    itT = np.ascontiguousarray(it.transpose(0, 2, 1)).astype(NP_FP8)
    hT = np.ascontiguousarray(
        f32(inputs["h"]).reshape(N_CORES, BL, H).transpose(0, 2, 1)
    ).astype(NP_BF16)
    h2T = np.ascontiguousarray(
        f32(inputs["h2"]).reshape(N_CORES, BL, H).transpose(0, 2, 1)
    ).astype(NP_BF16)

    trc = lambda a, d: np.ascontiguousarray(f32(a).T).astype(d)
    rep = {
        "wpm_t": trc(inputs["W_post_mu"], NP_FP8),
        "wps_t": trc(inputs["W_post_sigma"], NP_FP8),
        "wi2t_t": trc(inputs["W_I_to_theta"], NP_FP8),
        "wzh_t": trc(inputs["W_z_to_h"], NP_FP8),
        "wzh2_t": trc(inputs["W_z_to_h2"], NP_FP8),
        "wvip_t": trc(inputs["W_vip"], NP_FP8),
        "wt2z_t": trc(inputs["W_theta_to_z"], NP_FP8),
        "whh_t": trc(inputs["W_h_to_h"], NP_BF16),
        "wh2h2_t": trc(inputs["W_h2_to_h2"], NP_BF16),
        "wprs_t": trc(inputs["W_prior_sigma"], NP_BF16),
        "wprm_t": trc(inputs["W_prior_mu"], NP_BF16),
        "wrec1": np.ascontiguousarray(f32(inputs["W_rec1"])).astype(NP_BF16),
        "wrec2_t": trc(inputs["W_rec2"], NP_BF16),
        "bps": np.ascontiguousarray(f32(inputs["b_prior_sigma"]).reshape(1, L)),
    }
    return [
        {"packa": packa[i], "packb": packb[i], "itT": itT[i], "hT": hT[i],
         "h2T": h2T[i], **rep}
        for i in range(N_CORES)
    ]


def run(inputs, trace=False, **kw):
    nc = _get_program()
    in_maps = _prep_in_maps(inputs)
    res = run_bass_kernel_spmd(
        nc, in_maps, core_ids=list(range(N_CORES)), trace=trace, **kw
    )
    out = np.concatenate([res.results[i]["out"] for i in range(N_CORES)], axis=0)
    return out, res


def kernel(**inputs):
    out, _ = run(inputs)
    return out
